# revision 31
# baseline (speedup 1.0000x reference)
"""DCRNN (2-layer encoder/decoder DCGRU, N=512 nodes, B=32, U=64, K=2, 2 supports)
Trainium2 Bass/Tile kernel, data-parallel over batch across 8 NeuronCores.

Formulation: gconv(X) = sum_m T_m @ X @ W_m with T_m precomputed on host
(m=0 is the identity and is folded into stage 2 as a direct X @ W_0 matmul).
  stage 1 (dense):     A_m = X @ W_m, m=1..4  (X-chunk as lhsT -> node-major A)
  stage 2 (diffusion): out = X @ W_0 + sum_m (T_m A_m)^T, accumulated in PSUM,
                       bias+sigmoid/tanh fused into the PSUM->SBUF activation.
All matmul operands bf16 (or fp8e4m3 with DoubleRow for the diffusion stage:
two 128-row node chunks contracted per matmul at 2x rate). State tiles bf16.
Layout avoids every partition-shift copy:
  X0  [65,BI]  rows 0:64 h0,    row 64 x      (L0 gate lhsT)
  X0c [65,BI]  rows 0:64 r0*h0, row 64 x      (L0 cand lhsT)
  X1  [128,BI] rows 0:64 h0',   rows 64:128 h1        (L1 gate lhsT)
  X1c [128,BI] rows 0:64 h0',   rows 64:128 r1*h1     (L1 cand lhsT)
h1 and r1*h1 are written at partition base 64 directly by DVE (cross-base ok).
"""

import sys

sys.path.insert(0, "/opt/trn_rl_repo")

import numpy as np

import concourse.bass as bass
import concourse.mybir as mybir
import concourse.tile as tile
from concourse import bacc, bass_utils

# Model dims (fixed by the problem)
N = 512
T_ENC = 12
HOR = 12
U = 64
NM = 5  # diffusion matrices (I + 2 per support * 2 supports)
B = 32
NCORES = 8
BL = B // NCORES  # local batch = 4
BI = BL * N  # 2048: the (b, node) free dim
C0 = 1 + U  # 65 input channels, layer 0
C1 = U + U  # 128 input channels, layer 1
KCH = N // 128  # 4 node chunks

F32 = mybir.dt.float32
BF16 = mybir.dt.bfloat16
FP8 = mybir.dt.float8e4
AF = mybir.ActivationFunctionType
DR = mybir.MatmulPerfMode.DoubleRow
TSCALE = 128.0  # pre-scale on T & identity-W so fp8 T entries are normal-range

# stage-2 diffusion: gate = all-m fp8 DoubleRow; cand = m2,m4 bf16 (dominant
# terms) + m1,m3 fp8 DoubleRow (terms ~10% of output norm, fp8 error diluted)
import os as _os

LDW_OPT = _os.environ.get("LDW_OPT", "0") == "1"

_ldw_patched = False


def _patch_ldw_opt():
    global _ldw_patched
    if _ldw_patched or not LDW_OPT:
        return
    _ldw_patched = True
    orig = bass_utils.bir_verify_and_optimise

    def patched(tmpdir, inp="bir.json", outp="file.neff", arch=None, *, dve_root=None):
        import concourse.bass_utils as bu

        real_run = bu.run_command

        def run_hook(cmd, **kw):
            cmd = [
                c.replace("--enable-ldw-opt=false", "--enable-ldw-opt=true")
                for c in cmd
            ]
            return real_run(cmd, **kw)

        bu.run_command = run_hook
        try:
            return orig(tmpdir, inp, outp, arch, dve_root=dve_root)
        finally:
            bu.run_command = real_run

    bass_utils.bir_verify_and_optimise = patched


def _build_program(n_enc=T_ENC, n_dec=HOR, fold=True):
    _patch_ldw_opt()
    nc = bacc.Bacc("TRN2", target_bir_lowering=False, debug=False)

    # ---- DRAM I/O ----
    d_xenc = nc.dram_tensor("xenc", [n_enc, BI], BF16, kind="ExternalInput")
    d_tm = {}
    for s2 in ("fp8", "bf16"):
        dt_ = FP8 if s2 == "fp8" else BF16
        d_tm[s2] = nc.dram_tensor(
            f"tm_{s2}", [NM * KCH * 128, 512], dt_, kind="ExternalInput"
        )
    d_w = {}
    for pfx in ("e", "d"):
        for lyr, c_in in ((0, C0), (1, C1)):
            d_w[f"{pfx}wg{lyr}"] = nc.dram_tensor(
                f"{pfx}wg{lyr}", [c_in, NM * 2 * U], BF16, kind="ExternalInput"
            )
            d_w[f"{pfx}wc{lyr}"] = nc.dram_tensor(
                f"{pfx}wc{lyr}", [c_in, NM * U], BF16, kind="ExternalInput"
            )
            d_w[f"{pfx}bg{lyr}"] = nc.dram_tensor(
                f"{pfx}bg{lyr}", [2 * U, 1], F32, kind="ExternalInput"
            )
            d_w[f"{pfx}bc{lyr}"] = nc.dram_tensor(
                f"{pfx}bc{lyr}", [U, 1], F32, kind="ExternalInput"
            )
    if fold:
        d_w["dwg0f"] = nc.dram_tensor(
            "dwg0f", [C1, NM * 2 * U], BF16, kind="ExternalInput"
        )
    d_pw = nc.dram_tensor("pw", [U, 2], BF16, kind="ExternalInput")
    d_pb = nc.dram_tensor("pb", [1, 1], F32, kind="ExternalInput")
    d_out = nc.dram_tensor("outs", [n_dec, BI], BF16, kind="ExternalOutput")

    with tile.TileContext(nc) as tc:
        _body(tc, n_enc, n_dec, d_xenc, d_tm, d_w, d_pw, d_pb, d_out, fold)
    nc.compile()
    return nc


def _body(tc, n_enc, n_dec, d_xenc, d_tm, d_w, d_pw, d_pb, d_out, fold):
    nc = tc.nc
    consts = tc.alloc_tile_pool(name="consts", bufs=1)
    work = tc.alloc_tile_pool(name="work", bufs=1)
    gpool = tc.alloc_tile_pool(name="gpool", bufs=2)
    ag_pool = tc.alloc_tile_pool(name="agp", bufs=10)
    ac_pool = tc.alloc_tile_pool(name="acp", bufs=6)
    ps1 = tc.alloc_tile_pool(name="ps1", bufs=2, space="PSUM")
    ps2 = tc.alloc_tile_pool(name="ps2", bufs=2, space="PSUM")

    # ---- resident constants ----
    # tm layout: [128(p), m, kpair, j, 512]; node index = (kpair*2+j)*128 + p
    tm_sb = {}
    for s2 in ("fp8", "bf16"):
        dt_ = FP8 if s2 == "fp8" else BF16
        t = consts.tile([128, NM, 2, 2, 512], dt_, name=f"tm_sb_{s2}")
        tm_sb[s2] = t
        for m in range(NM):
            for k in range(KCH):
                row = (m * KCH + k) * 128
                nc.sync.dma_start(
                    out=t[:, m, k // 2, k % 2, :], in_=d_tm[s2][row : row + 128, :]
                )

    w_sb = {}
    for key, dt_ in d_w.items():
        shape = list(dt_.shape)
        sb_dt = BF16 if key[1] == "w" else F32
        w_sb[key] = consts.tile(shape, sb_dt, name=f"sb_{key}")
        nc.sync.dma_start(out=w_sb[key][:, :], in_=dt_[:, :])
    pw_sb = consts.tile([128, 2], BF16, name="pw_sb")
    nc.sync.dma_start(out=pw_sb[64:128, :], in_=d_pw[:, :])
    pb_sb = consts.tile([1, 1], F32, name="pb_sb")
    nc.sync.dma_start(out=pb_sb, in_=d_pb[:, :])

    # ---- persistent state ----
    X0 = work.tile([C0, BI], BF16, name="X0")  # [h0 ; x]
    X0c = work.tile([C0, BI], BF16, name="X0c")  # [r0*h0 ; x]
    X1 = work.tile([C1, BI], BF16, name="X1")  # [h0 ; h1]
    X1c = work.tile([C1, BI], BF16, name="X1c")  # [h0 ; r1*h1]

    nc.gpsimd.memset(X0[0:U, :], 0.0)
    nc.gpsimd.memset(X0c[0:U, :], 0.0)
    nc.gpsimd.memset(X1[:, :], 0.0)
    nc.gpsimd.memset(X1c[:, :], 0.0)

    tc.strict_bb_all_engine_barrier()

    def cell_phases(lyr, Xg, Xc, cg, cc, wg, bg, wc, bc, h_src, r_dst, h_dst, post):
        """One DCGRU cell, split into per-batch-pair phases.

        Layer-l elementwise state lives at partition base l*64 so every
        two-tensor DVE op has matching input bases (h1 sits at rows 64:128 of
        X1).  The gate output layout is [r; u] for layer 0 and [u; r] for
        layer 1 (weights pre-flipped on host), so r shares a base with h; the
        u half is moved across with one single-src copy per pair.
        """
        gdt = FP8
        sl = slice(lyr * U, (lyr + 1) * U)  # this layer's partition rows
        u_src = slice(U, 2 * U) if lyr == 0 else slice(0, U)  # u half of RU
        r_src = slice(0, U) if lyr == 0 else slice(U, 2 * U)  # r half of RU
        RU = gpool.tile([2 * U, BI], BF16, tag="RU", name="RU", bufs=2)
        Uu = gpool.tile([2 * U, BI], BF16, tag="Uu", name="Uu", bufs=2)
        Wu = gpool.tile([2 * U, BI], BF16, tag="Wu", name="Wu", bufs=2)
        uh = gpool.tile([2 * U, BI], BF16, tag="uh", name="uh", bufs=2)
        Ct = gpool.tile([2 * U, BI], BF16, tag="Ct", name="Ct", bufs=2)
        wct = gpool.tile([2 * U, BI], BF16, tag="wct", name="wct", bufs=2)
        ag = {}
        ac = {}
        cacc = [None]

        def gate_phase(p):
            # stage 1: A_m = X @ Wg_m for m=1..4 (m=0 folded into stage 2).
            # Two node chunks share a 2-bank PSUM pair tile -> one wide copy.
            for b in (2 * p, 2 * p + 1):
                for kp in range(2):
                    pg = ps1.tile([128, 2, 512], F32, tag="s1g", name="pg")
                    for j in range(2):
                        k = kp * 2 + j
                        lhsT = Xg[0:cg, b * N + k * 128 : b * N + (k + 1) * 128]
                        nc.tensor.matmul(
                            pg[:, j, :], lhsT, wg[:, 128:640], start=True, stop=True
                        )
                    a = ag_pool.tile(
                        [128, 2, NM - 1, 128], gdt, tag="ag", name="ag"
                    )
                    ag[(b, kp)] = a
                    # split the PSUM->SBUF A copies across Scalar and Vector
                    # so neither queue rate-limits the gate phase
                    srcv = pg.rearrange("p j (m c) -> p j m c", m=NM - 1)
                    if (b + kp) % 2 == 0:
                        nc.scalar.copy(out=a[:, :, :, :], in_=srcv)
                    else:
                        nc.vector.tensor_copy(out=a[:, :, :, :], in_=srcv)
            # stage 2: acc = X @ Wg_0 + sum_{m>0} (T_m A_m)^T, fused sigmoid.
            # The two batches of the pair share a 2-bank acc -> one wide act.
            accp = ps2.tile([128, 2, 512], F32, tag="s2", name="accg")
            for half, b in enumerate((2 * p, 2 * p + 1)):
                acc = accp[:, half, :]
                nc.tensor.matmul(
                    acc,
                    wg[:, 0:128],
                    Xg[0:cg, b * N : (b + 1) * N],
                    start=True,
                    stop=False,
                )
                for m in range(1, NM):
                    for kp in range(2):
                        nc.tensor.matmul(
                            acc,
                            ag[(b, kp)][:, :, m - 1, :],
                            tm_sb["fp8"][:, m, kp],
                            start=False,
                            stop=(m == NM - 1 and kp == 1),
                            perf_mode=DR,
                        )
            # per-half acts + DVE chain: each batch's sigmoid -> r*h releases
            # that batch's cand stage-1 without waiting for the pair
            for half, b in enumerate((2 * p, 2 * p + 1)):
                bcols = slice(b * N, (b + 1) * N)
                nc.scalar.activation(
                    out=RU[:, bcols], in_=accp[:, half, :],
                    func=AF.Sigmoid, bias=bg[:, 0:1], scale=1.0 / TSCALE,
                )
                # r*h first: it gates the cand phase's stage-1 (critical path)
                nc.vector.tensor_mul(
                    out=r_dst(bcols), in0=RU[r_src, bcols], in1=h_src(bcols)
                )
            pcols = slice(2 * p * N, 2 * (p + 1) * N)
            # move u to this layer's partition rows (single-src cross-base copy)
            nc.vector.tensor_copy(out=Uu[sl, pcols], in_=RU[u_src, pcols])
            nc.vector.tensor_mul(
                out=uh[sl, pcols], in0=Uu[sl, pcols], in1=h_src(pcols)
            )
            nc.vector.tensor_scalar(
                out=Wu[sl, pcols], in0=Uu[sl, pcols],
                scalar1=-1.0, scalar2=1.0,
                op0=mybir.AluOpType.mult, op1=mybir.AluOpType.add,
            )

        def cand_phase(p):
            # stage 1: m=1..4 only; all 4 chunks of one batch share a 2-bank
            # pair tile (256 cols each) -> one wide copy per batch
            for half, b in enumerate((2 * p, 2 * p + 1)):
                pc = ps1.tile([128, 2, 512], F32, tag="s1g", name="pc")
                for k in range(KCH):
                    lhsT = Xc[0:cc, b * N + k * 128 : b * N + (k + 1) * 128]
                    nc.tensor.matmul(
                        pc[:, k // 2, (k % 2) * 256 : (k % 2) * 256 + 256],
                        lhsT,
                        wc[:, U : NM * U],
                        start=True,
                        stop=True,
                        skip_group_check=(k % 2 == 1),
                    )
                if half == 0:
                    # split A by m: m2,m4 -> bf16 (dominant), m1,m3 -> fp8 DR
                    ac[p] = (
                        ac_pool.tile([128, 2, 2, 2, 2, U], BF16, tag="a24", name="a24"),
                        ac_pool.tile([128, 2, 2, 2, 2, U], FP8, tag="a13", name="a13"),
                    )
                # pc free layout (kp, j, m, u); m = 2*m2 + mm: mm=0 -> m13,
                # mm=1 -> m24 (m index here is m-1 for m=1..4)
                srcv = pc.rearrange(
                    "p kp (j m2 mm u) -> p kp j m2 mm u", j=2, m2=2, mm=2
                )
                # balance the two casts across Scalar and Vector per half
                e24 = nc.vector if half == 0 else nc.scalar
                e13 = nc.scalar if half == 0 else nc.vector
                (e24.tensor_copy if e24 is nc.vector else e24.copy)(
                    out=ac[p][0][:, :, :, :, half, :], in_=srcv[:, :, :, :, 1, :]
                )
                (e13.tensor_copy if e13 is nc.vector else e13.copy)(
                    out=ac[p][1][:, :, :, :, half, :], in_=srcv[:, :, :, :, 0, :]
                )
            # stage 2: identity fold (col-tiled pair) + diffusion, fused tanh
            # (both p's share one 2-bank pair tile, one bank each)
            if cacc[0] is None:
                cacc[0] = ps2.tile([128, 2, 512], F32, tag="s2", name="accc")
            acc = cacc[0][:, p, :]
            for half in range(2):
                b = 2 * p + half
                nc.tensor.matmul(
                    acc[half * U : (half + 1) * U, :],
                    wc[:, 0:U],
                    Xc[0:cc, b * N : (b + 1) * N],
                    start=True,  # per-partition zero region: each half starts its own rows
                    stop=False,
                    tile_position=(0, half * U),
                    # sim's group tracker isn't partition-base-aware; half 1 would
                    # falsely collide with half 0's pending group
                    skip_group_check=True,
                )
            for mi, m in enumerate((2, 4)):
                for k in range(KCH):
                    nc.tensor.matmul(
                        acc,
                        ac[p][0][:, k // 2, k % 2, mi, :, :],
                        tm_sb["bf16"][:, m, k // 2, k % 2, :],
                        start=False,
                        stop=False,
                        skip_group_check=True,
                    )
            for mi, m in enumerate((1, 3)):
                for kp in range(2):
                    nc.tensor.matmul(
                        acc,
                        ac[p][1][:, kp, :, mi, :, :],
                        tm_sb["fp8"][:, m, kp],
                        start=False,
                        stop=(mi == 1 and kp == 1),
                        perf_mode=DR,
                        skip_group_check=True,
                    )
            # per-half tanh + blend: batch b's h lands without waiting for
            # the pair, releasing the next phase's stage-1 per batch
            for half in range(2):
                b = 2 * p + half
                bcols = slice(b * N, (b + 1) * N)
                nc.scalar.activation(
                    out=Ct[sl, bcols],
                    in_=acc[half * U : (half + 1) * U, :],
                    func=AF.Tanh, bias=bc[:, 0:1], scale=1.0 / TSCALE,
                )
                # h_new = u*h + (1-u)*c
                nc.vector.tensor_mul(
                    out=wct[sl, bcols], in0=Wu[sl, bcols], in1=Ct[sl, bcols]
                )
                nc.vector.tensor_add(
                    out=h_dst(bcols), in0=uh[sl, bcols], in1=wct[sl, bcols]
                )
            post(p)

        return gate_phase, cand_phase

    def l0_h_src(cols):
        return X0[0:U, cols]

    def l0_r_dst(cols):
        return X0c[0:U, cols]

    def l0_h_dst(cols):
        # write h0' straight into X1 so g1's stage-1 doesn't wait on a copy
        return X1[0:U, cols]

    def l0_post(p):
        # fan h0' out to the other consumers, off the g1 critical path:
        # X1c feeds c1's stage-1 (medium slack), X0 feeds the NEXT step's g0
        # (GpSimd measured 5x slower than DVE for these copies)
        pcols = slice(2 * p * N, 2 * (p + 1) * N)
        nc.gpsimd.tensor_copy(out=X1c[0:U, pcols], in_=X1[0:U, pcols])
        nc.gpsimd.tensor_copy(out=X0[0:U, pcols], in_=X1[0:U, pcols])

    def l1_h_src(cols):
        return X1[U:C1, cols]

    def l1_r_dst(cols):
        return X1c[U:C1, cols]

    def l1_h_dst(cols):
        return X1[U:C1, cols]

    def l1_post(p):
        pass

    def proj_phase(p, feed_cand=False):
        # projection for pair p: out = h1 . pw + pb (row 0 of pp)
        ppp = ps2.tile([128, 2, 512], F32, tag="s2", name="pp")
        for q in (2 * p, 2 * p + 1):
            pp = ppp[:, q % 2, :]
            nc.tensor.matmul(
                pp[0:2, :],
                pw_sb[64:128, :],
                X1[U:C1, q * 512 : (q + 1) * 512],
                start=True,
                stop=True,
            )
        # X0 row first (gates the next step's g0 stage-1), X0c after (only
        # read by the later cand stage-1)
        for q in (2 * p, 2 * p + 1):
            nc.scalar.activation(
                out=X0[U:C0, q * 512 : (q + 1) * 512],
                in_=ppp[0:1, q % 2, :],
                func=AF.Identity,
                bias=pb_sb[:, 0:1],
                scale=1.0,
            )
        if feed_cand:
            # decoder feedback: write the cand-path x row directly too,
            # replacing a serial 1-partition [1, BI] copy on the DVE
            for q in (2 * p, 2 * p + 1):
                nc.scalar.activation(
                    out=X0c[U:C0, q * 512 : (q + 1) * 512],
                    in_=ppp[0:1, q % 2, :],
                    func=AF.Identity,
                    bias=pb_sb[:, 0:1],
                    scale=1.0,
                )

    def build_step(pfx, fold0=False):
        if fold0:
            # decoder t>=1: x = h1.pw (+pb==0) folded into the L0 gate
            # weights, so the gate reads [h0; h1] from X1 and never waits on
            # the projection chain
            xg0, cg0, wg0 = X1, C1, w_sb["dwg0f"]
        else:
            xg0, cg0, wg0 = X0, C0, w_sb[f"{pfx}wg0"]
        g0, c0 = cell_phases(
            0, xg0, X0c, cg0, C0,
            wg0, w_sb[f"{pfx}bg0"], w_sb[f"{pfx}wc0"],
            w_sb[f"{pfx}bc0"], l0_h_src, l0_r_dst, l0_h_dst, l0_post,
        )
        g1, c1 = cell_phases(
            1, X1, X1c, C1, C1,
            w_sb[f"{pfx}wg1"], w_sb[f"{pfx}bg1"], w_sb[f"{pfx}wc1"],
            w_sb[f"{pfx}bc1"], l1_h_src, l1_r_dst, l1_h_dst, l1_post,
        )
        return g0, c0, g1, c1

    def stage_x(t):
        def emit(dst):
            nc.sync.dma_start(out=dst[U:C0, :], in_=d_xenc[t : t + 1, :])
        return emit

    def zero_x(dst):
        nc.vector.memset(dst[U:C0, :], 0.0)

    # Software-pipelined emission with a 1-phase skew: the trailing cand
    # phase (and decoder projection) of step t interleaves with step t+1's
    # gate matmuls so the PE never drains at a step boundary.
    # steps: list of (pfx, x_hook or None, dec_t or None)
    steps = []
    for t in range(n_enc):
        if t == 0:
            hook = None  # x_0 staged before the loop
        else:
            hook = stage_x(t)
        steps.append(("e", hook, None))
    steps.append(("d", (lambda dst: zero_x(dst)), 0))
    for t in range(1, n_dec):
        steps.append(("d", None, t))

    stage_x(0)(X0)
    stage_x(0)(X0c)
    pending = None  # (c1, dec_t) of the previous step
    for pfx, x_hook, dec_t in steps:
        fold0 = fold and dec_t is not None and x_hook is None
        g0, c0, g1, c1 = build_step(pfx, fold0)
        # x for THIS step must land before this step's g0/c0 read it; the
        # hook writes row 64 only, after the previous step's readers.
        if x_hook is not None:
            x_hook(X0)
        if pending is not None:
            # both cand-L1 pairs first: pair-1 PE work covers pair-0's
            # tanh+blend latency, then proj (which needs the blended h1)
            pc1, pdec = pending
            pc1(0)
            pc1(1)
            if pdec is not None:
                feed = dec_t is not None and x_hook is None
                proj_phase(0, feed_cand=feed)
                proj_phase(1, feed_cand=feed)
                nc.sync.dma_start(out=d_out[pdec : pdec + 1, :], in_=X0[U:C0, :])
        g0(0)
        g0(1)
        if x_hook is not None:
            x_hook(X0c)
        c0(0); c0(1)
        g1(0); g1(1)
        pending = (c1, dec_t)

    pc1, pdec = pending
    pc1(0); pc1(1)
    proj_phase(0); proj_phase(1)
    nc.sync.dma_start(out=d_out[pdec : pdec + 1, :], in_=X0[U:C0, :])

    for pool in (ps2, ps1, ac_pool, ag_pool, gpool, work, consts):
        pool.release()


# --------------------------------------------------------------------------
# host-side packing
# --------------------------------------------------------------------------
def _prep_shared(inputs):
    bf = mybir.dt.np(BF16)
    f8 = mybir.dt.np(FP8)
    sup = np.asarray(inputs["supports"], np.float64)
    eye = np.eye(N, dtype=np.float64)
    tms = [
        eye,
        sup[0],
        2.0 * (sup[0] @ sup[0]) - eye,
        sup[1],
        2.0 * (sup[1] @ sup[1]) - eye,
    ]
    # T (and the identity W blocks) are pre-scaled by TSCALE so fp8 entries
    # land in e4m3's normal range; the PSUM->SBUF activation undoes it via
    # scale=1/TSCALE.  |T|max ~1.05 -> 134 < 448, safe.
    tmats = np.stack([t.T * TSCALE for t in tms]).astype(np.float32)
    tmats = tmats.reshape(NM * KCH * 128, 512)

    shared = {}
    for s2 in ("fp8", "bf16"):
        dt_ = f8 if s2 == "fp8" else bf
        shared[f"tm_{s2}"] = np.ascontiguousarray(tmats.astype(dt_))
    for pfx, name in (("e", "enc"), ("d", "dec")):
        for lyr, c_in in ((0, C0), (1, C1)):
            wg = np.asarray(inputs[f"{name}{lyr}_Wg"], np.float32).reshape(
                c_in, NM * 2 * U
            )
            wc = np.asarray(inputs[f"{name}{lyr}_Wc"], np.float32).reshape(
                c_in, NM * U
            )
            bg = np.asarray(inputs[f"{name}{lyr}_bg"], np.float32)
            bc = np.asarray(inputs[f"{name}{lyr}_bc"], np.float32)
            # scale the identity (m=0) block to match the TSCALE'd T terms
            # (copy: the reshaped views alias the caller's input arrays)
            wg = wg.copy()
            wc = wc.copy()
            wg[:, 0 : 2 * U] *= TSCALE
            wc[:, 0:U] *= TSCALE
            if lyr == 0:
                perm = np.r_[1:c_in, 0]  # rows [h..., x]
                wg = wg[perm]
                wc = wc[perm]
            else:
                # layer-1 gate layout is [u; r] (see cell_phases): swap the
                # r/u column halves inside each m block, and the bias halves
                wg = np.ascontiguousarray(
                    wg.reshape(c_in, NM, 2, U)[:, :, ::-1, :].reshape(c_in, NM * 2 * U)
                )
                bg = np.concatenate([bg[U:], bg[:U]])
            shared[f"{pfx}wg{lyr}"] = np.ascontiguousarray(wg.astype(bf))
            if pfx == "d" and lyr == 0:
                pw_f = np.asarray(inputs["proj_W"], np.float64).reshape(U, 1)
                fold = np.vstack([wg[0:U], pw_f @ wg[U : U + 1]]).astype(np.float32)
                shared["dwg0f"] = np.ascontiguousarray(fold.astype(bf))
            shared[f"{pfx}wc{lyr}"] = np.ascontiguousarray(wc.astype(bf))
            shared[f"{pfx}bg{lyr}"] = np.ascontiguousarray(bg.reshape(2 * U, 1))
            shared[f"{pfx}bc{lyr}"] = np.ascontiguousarray(bc.reshape(U, 1))
    pw = np.asarray(inputs["proj_W"], np.float32).reshape(U, 1)
    shared["pw"] = np.ascontiguousarray(
        np.concatenate([pw, np.zeros((U, 1), np.float32)], axis=1).astype(bf)
    )
    shared["pb"] = np.asarray(inputs["proj_b"], np.float32).reshape(1, 1)
    return shared


def _make_in_maps(inputs, n_enc=T_ENC):
    bf = mybir.dt.np(BF16)
    shared = _prep_shared(inputs)
    x = np.asarray(inputs["inputs"], np.float32)  # (T, B, N)
    in_maps = []
    for c in range(NCORES):
        m = dict(shared)
        m["xenc"] = np.ascontiguousarray(
            x[:n_enc, c * BL : (c + 1) * BL, :].reshape(n_enc, BI).astype(bf)
        )
        in_maps.append(m)
    return in_maps


_PROG_CACHE = {}


def _get_program(n_enc=T_ENC, n_dec=HOR, fold=True):
    key = (n_enc, n_dec, fold)
    if key not in _PROG_CACHE:
        _PROG_CACHE[key] = _build_program(n_enc, n_dec, fold)
    return _PROG_CACHE[key]


def _run(inputs, n_enc=T_ENC, n_dec=HOR, **kw):
    fold = bool(np.allclose(np.asarray(inputs["proj_b"], np.float64), 0.0))
    nc = _get_program(n_enc, n_dec, fold)
    in_maps = _make_in_maps(inputs, n_enc)
    if not fold:
        for m in in_maps:
            m.pop("dwg0f", None)
    res = bass_utils.run_bass_kernel_spmd(nc, in_maps, core_ids=list(range(NCORES)), **kw)
    out = np.empty((n_dec, B, N), np.float32)
    for c in range(NCORES):
        out[:, c * BL : (c + 1) * BL, :] = (
            res.results[c]["outs"].astype(np.float32).reshape(n_dec, BL, N)
        )
    return out.reshape(n_dec, B, N), res


def kernel(**inputs) -> np.ndarray:
    out, _ = _run(inputs)
    return out.reshape(HOR, B, N)



# revision 32
# speedup vs baseline: 1.0703x; 1.0703x over previous
"""DCRNN (2-layer encoder/decoder DCGRU, N=512 nodes, B=32, U=64, K=2, 2 supports)
Trainium2 Bass/Tile kernel, data-parallel over batch across 8 NeuronCores.

Formulation: gconv(X) = sum_m T_m @ X @ W_m with T_m precomputed on host
(m=0 is the identity and is folded into stage 2 as a direct X @ W_0 matmul).
  stage 1 (dense):     A_m = X @ W_m, m=1..4  (X-chunk as lhsT -> node-major A)
  stage 2 (diffusion): out = X @ W_0 + sum_m (T_m A_m)^T, accumulated in PSUM,
                       bias+sigmoid/tanh fused into the PSUM->SBUF activation.
All matmul operands bf16 (or fp8e4m3 with DoubleRow for the diffusion stage:
two 128-row node chunks contracted per matmul at 2x rate). State tiles bf16.
Layout avoids every partition-shift copy:
  X0  [65,BI]  rows 0:64 h0,    row 64 x      (L0 gate lhsT)
  X0c [65,BI]  rows 0:64 r0*h0, row 64 x      (L0 cand lhsT)
  X1  [128,BI] rows 0:64 h0',   rows 64:128 h1        (L1 gate lhsT)
  X1c [128,BI] rows 0:64 h0',   rows 64:128 r1*h1     (L1 cand lhsT)
h1 and r1*h1 are written at partition base 64 directly by DVE (cross-base ok).
"""

import sys

sys.path.insert(0, "/opt/trn_rl_repo")

import numpy as np

import concourse.bass as bass
import concourse.mybir as mybir
import concourse.tile as tile
from concourse import bacc, bass_utils

# Model dims (fixed by the problem)
N = 512
T_ENC = 12
HOR = 12
U = 64
NM = 5  # diffusion matrices (I + 2 per support * 2 supports)
B = 32
NCORES = 8
BL = B // NCORES  # local batch = 4
BI = BL * N  # 2048: the (b, node) free dim
C0 = 1 + U  # 65 input channels, layer 0
C1 = U + U  # 128 input channels, layer 1
KCH = N // 128  # 4 node chunks

F32 = mybir.dt.float32
BF16 = mybir.dt.bfloat16
FP8 = mybir.dt.float8e4
AF = mybir.ActivationFunctionType
DR = mybir.MatmulPerfMode.DoubleRow
TSCALE = 128.0  # pre-scale on T & identity-W so fp8 T entries are normal-range

# stage-2 diffusion: gate = all-m fp8 DoubleRow; cand = m2,m4 bf16 (dominant
# terms) + m1,m3 fp8 DoubleRow (terms ~10% of output norm, fp8 error diluted)
import os as _os

LDW_OPT = _os.environ.get("LDW_OPT", "0") == "1"

_ldw_patched = False


def _patch_ldw_opt():
    global _ldw_patched
    if _ldw_patched or not LDW_OPT:
        return
    _ldw_patched = True
    orig = bass_utils.bir_verify_and_optimise

    def patched(tmpdir, inp="bir.json", outp="file.neff", arch=None, *, dve_root=None):
        import concourse.bass_utils as bu

        real_run = bu.run_command

        def run_hook(cmd, **kw):
            cmd = [
                c.replace("--enable-ldw-opt=false", "--enable-ldw-opt=true")
                for c in cmd
            ]
            return real_run(cmd, **kw)

        bu.run_command = run_hook
        try:
            return orig(tmpdir, inp, outp, arch, dve_root=dve_root)
        finally:
            bu.run_command = real_run

    bass_utils.bir_verify_and_optimise = patched


def _build_program(n_enc=T_ENC, n_dec=HOR, fold=True):
    _patch_ldw_opt()
    nc = bacc.Bacc("TRN2", target_bir_lowering=False, debug=False)

    # ---- DRAM I/O ----
    d_xenc = nc.dram_tensor("xenc", [n_enc, BI], BF16, kind="ExternalInput")
    d_tm = {}
    for s2 in ("fp8", "bf16"):
        dt_ = FP8 if s2 == "fp8" else BF16
        d_tm[s2] = nc.dram_tensor(
            f"tm_{s2}", [NM * KCH * 128, 512], dt_, kind="ExternalInput"
        )
    d_w = {}
    for pfx in ("e", "d"):
        for lyr, c_in in ((0, C0), (1, C1)):
            d_w[f"{pfx}wg{lyr}"] = nc.dram_tensor(
                f"{pfx}wg{lyr}", [c_in, NM * 2 * U], BF16, kind="ExternalInput"
            )
            d_w[f"{pfx}wc{lyr}"] = nc.dram_tensor(
                f"{pfx}wc{lyr}", [c_in, NM * U], BF16, kind="ExternalInput"
            )
            d_w[f"{pfx}bg{lyr}"] = nc.dram_tensor(
                f"{pfx}bg{lyr}", [2 * U, 1], F32, kind="ExternalInput"
            )
            d_w[f"{pfx}bc{lyr}"] = nc.dram_tensor(
                f"{pfx}bc{lyr}", [U, 1], F32, kind="ExternalInput"
            )
    if fold:
        d_w["dwg0f"] = nc.dram_tensor(
            "dwg0f", [C1, NM * 2 * U], BF16, kind="ExternalInput"
        )
    d_pw = nc.dram_tensor("pw", [U, 2], BF16, kind="ExternalInput")
    d_pb = nc.dram_tensor("pb", [1, 1], F32, kind="ExternalInput")
    d_out = nc.dram_tensor("outs", [n_dec, BI], BF16, kind="ExternalOutput")

    with tile.TileContext(nc) as tc:
        _body(tc, n_enc, n_dec, d_xenc, d_tm, d_w, d_pw, d_pb, d_out, fold)
    nc.compile()
    return nc


def _body(tc, n_enc, n_dec, d_xenc, d_tm, d_w, d_pw, d_pb, d_out, fold):
    nc = tc.nc
    consts = tc.alloc_tile_pool(name="consts", bufs=1)
    work = tc.alloc_tile_pool(name="work", bufs=1)
    gpool = tc.alloc_tile_pool(name="gpool", bufs=2)
    ag_pool = tc.alloc_tile_pool(name="agp", bufs=10)
    ac_pool = tc.alloc_tile_pool(name="acp", bufs=6)
    ps1 = tc.alloc_tile_pool(name="ps1", bufs=2, space="PSUM")
    ps2 = tc.alloc_tile_pool(name="ps2", bufs=2, space="PSUM")

    # ---- resident constants ----
    # tm layout: [128(p), m, kpair, j, 512]; node index = (kpair*2+j)*128 + p
    tm_sb = {}
    for s2 in ("fp8", "bf16"):
        dt_ = FP8 if s2 == "fp8" else BF16
        t = consts.tile([128, NM, 2, 2, 512], dt_, name=f"tm_sb_{s2}")
        tm_sb[s2] = t
        for m in range(NM):
            for k in range(KCH):
                row = (m * KCH + k) * 128
                nc.sync.dma_start(
                    out=t[:, m, k // 2, k % 2, :], in_=d_tm[s2][row : row + 128, :]
                )

    w_sb = {}
    for key, dt_ in d_w.items():
        shape = list(dt_.shape)
        sb_dt = BF16 if key[1] == "w" else F32
        w_sb[key] = consts.tile(shape, sb_dt, name=f"sb_{key}")
        nc.sync.dma_start(out=w_sb[key][:, :], in_=dt_[:, :])
    pw_sb = consts.tile([128, 2], BF16, name="pw_sb")
    nc.sync.dma_start(out=pw_sb[64:128, :], in_=d_pw[:, :])
    pb_sb = consts.tile([1, 1], F32, name="pb_sb")
    nc.sync.dma_start(out=pb_sb, in_=d_pb[:, :])

    # ---- persistent state ----
    X0 = work.tile([C0, BI], BF16, name="X0")  # [h0 ; x]
    X0c = work.tile([C0, BI], BF16, name="X0c")  # [r0*h0 ; x]
    X1 = work.tile([C1, BI], BF16, name="X1")  # [h0 ; h1]
    X1c = work.tile([C1, BI], BF16, name="X1c")  # [h0 ; r1*h1]

    nc.gpsimd.memset(X0[0:U, :], 0.0)
    nc.gpsimd.memset(X0c[0:U, :], 0.0)
    nc.gpsimd.memset(X1[:, :], 0.0)
    nc.gpsimd.memset(X1c[:, :], 0.0)

    tc.strict_bb_all_engine_barrier()

    def cell_phases(lyr, Xg, Xc, cg, cc, wg, bg, wc, bc, h_src, r_dst, h_dst, post):
        """One DCGRU cell, split into per-batch-pair phases.

        Layer-l elementwise state lives at partition base l*64 so every
        two-tensor DVE op has matching input bases (h1 sits at rows 64:128 of
        X1).  The gate output layout is [r; u] for layer 0 and [u; r] for
        layer 1 (weights pre-flipped on host), so r shares a base with h; the
        u half is moved across with one single-src copy per pair.
        """
        gdt = FP8
        sl = slice(lyr * U, (lyr + 1) * U)  # this layer's partition rows
        u_src = slice(U, 2 * U) if lyr == 0 else slice(0, U)  # u half of RU
        r_src = slice(0, U) if lyr == 0 else slice(U, 2 * U)  # r half of RU
        RU = gpool.tile([2 * U, BI], BF16, tag="RU", name="RU", bufs=2)
        Uu = gpool.tile([2 * U, BI], BF16, tag="Uu", name="Uu", bufs=2)
        Wu = gpool.tile([2 * U, BI], BF16, tag="Wu", name="Wu", bufs=2)
        uh = gpool.tile([2 * U, BI], BF16, tag="uh", name="uh", bufs=2)
        Ct = gpool.tile([2 * U, BI], BF16, tag="Ct", name="Ct", bufs=2)
        wct = gpool.tile([2 * U, BI], BF16, tag="wct", name="wct", bufs=2)
        ag = {}
        ac = {}
        cacc = [None]

        def gate_phase(p):
            # stage 1: A_m = X @ Wg_m for m=1..4 (m=0 folded into stage 2).
            # Two node chunks share a 2-bank PSUM pair tile -> one wide copy.
            for b in (2 * p, 2 * p + 1):
                for kp in range(2):
                    pg = ps1.tile([128, 2, 512], F32, tag="s1g", name="pg")
                    for j in range(2):
                        k = kp * 2 + j
                        lhsT = Xg[0:cg, b * N + k * 128 : b * N + (k + 1) * 128]
                        nc.tensor.matmul(
                            pg[:, j, :], lhsT, wg[:, 128:640], start=True, stop=True
                        )
                    a = ag_pool.tile(
                        [128, 2, NM - 1, 128], gdt, tag="ag", name="ag"
                    )
                    ag[(b, kp)] = a
                    # split the PSUM->SBUF A copies across Scalar and Vector
                    # so neither queue rate-limits the gate phase
                    srcv = pg.rearrange("p j (m c) -> p j m c", m=NM - 1)
                    if (b + kp) % 2 == 0:
                        nc.scalar.copy(out=a[:, :, :, :], in_=srcv)
                    else:
                        nc.vector.tensor_copy(out=a[:, :, :, :], in_=srcv)
            # stage 2: acc = X @ Wg_0 + sum_{m>0} (T_m A_m)^T, fused sigmoid.
            # The two batches of the pair share a 2-bank acc -> one wide act.
            accp = ps2.tile([128, 2, 512], F32, tag="s2", name="accg")
            for half, b in enumerate((2 * p, 2 * p + 1)):
                acc = accp[:, half, :]
                nc.tensor.matmul(
                    acc,
                    wg[:, 0:128],
                    Xg[0:cg, b * N : (b + 1) * N],
                    start=True,
                    stop=False,
                )
                for m in range(1, NM):
                    for kp in range(2):
                        nc.tensor.matmul(
                            acc,
                            ag[(b, kp)][:, :, m - 1, :],
                            tm_sb["fp8"][:, m, kp],
                            start=False,
                            stop=(m == NM - 1 and kp == 1),
                            perf_mode=DR,
                        )
            # per-half acts + DVE chain: each batch's sigmoid -> r*h releases
            # that batch's cand stage-1 without waiting for the pair
            for half, b in enumerate((2 * p, 2 * p + 1)):
                bcols = slice(b * N, (b + 1) * N)
                nc.scalar.activation(
                    out=RU[:, bcols], in_=accp[:, half, :],
                    func=AF.Sigmoid, bias=bg[:, 0:1], scale=1.0 / TSCALE,
                )
                # r*h first: it gates the cand phase's stage-1 (critical path)
                nc.vector.tensor_mul(
                    out=r_dst(bcols), in0=RU[r_src, bcols], in1=h_src(bcols)
                )
            pcols = slice(2 * p * N, 2 * (p + 1) * N)
            # move u to this layer's partition rows (single-src cross-base copy)
            nc.vector.tensor_copy(out=Uu[sl, pcols], in_=RU[u_src, pcols])
            nc.vector.tensor_mul(
                out=uh[sl, pcols], in0=Uu[sl, pcols], in1=h_src(pcols)
            )
            nc.vector.tensor_scalar(
                out=Wu[sl, pcols], in0=Uu[sl, pcols],
                scalar1=-1.0, scalar2=1.0,
                op0=mybir.AluOpType.mult, op1=mybir.AluOpType.add,
            )

        def cand_phase(p):
            # stage 1: m=1..4 only; all 4 chunks of one batch share a 2-bank
            # pair tile (256 cols each) -> one wide copy per batch
            for half, b in enumerate((2 * p, 2 * p + 1)):
                pc = ps1.tile([128, 2, 512], F32, tag="s1g", name="pc")
                for k in range(KCH):
                    lhsT = Xc[0:cc, b * N + k * 128 : b * N + (k + 1) * 128]
                    nc.tensor.matmul(
                        pc[:, k // 2, (k % 2) * 256 : (k % 2) * 256 + 256],
                        lhsT,
                        wc[:, U : NM * U],
                        start=True,
                        stop=True,
                        skip_group_check=(k % 2 == 1),
                    )
                if half == 0:
                    # split A by m: m2,m4 -> bf16 (dominant), m1,m3 -> fp8 DR
                    ac[p] = (
                        ac_pool.tile([128, 2, 2, 2, 2, U], BF16, tag="a24", name="a24"),
                        ac_pool.tile([128, 2, 2, 2, 2, U], FP8, tag="a13", name="a13"),
                    )
                # pc free layout (kp, j, m, u); m = 2*m2 + mm: mm=0 -> m13,
                # mm=1 -> m24 (m index here is m-1 for m=1..4)
                srcv = pc.rearrange(
                    "p kp (j m2 mm u) -> p kp j m2 mm u", j=2, m2=2, mm=2
                )
                # balance the two casts across Scalar and Vector per half
                e24 = nc.vector if half == 0 else nc.scalar
                e13 = nc.scalar if half == 0 else nc.vector
                (e24.tensor_copy if e24 is nc.vector else e24.copy)(
                    out=ac[p][0][:, :, :, :, half, :], in_=srcv[:, :, :, :, 1, :]
                )
                (e13.tensor_copy if e13 is nc.vector else e13.copy)(
                    out=ac[p][1][:, :, :, :, half, :], in_=srcv[:, :, :, :, 0, :]
                )
            # stage 2: identity fold (col-tiled pair) + diffusion, fused tanh
            # (both p's share one 2-bank pair tile, one bank each)
            if cacc[0] is None:
                cacc[0] = ps2.tile([128, 2, 512], F32, tag="s2", name="accc")
            acc = cacc[0][:, p, :]
            for half in range(2):
                b = 2 * p + half
                nc.tensor.matmul(
                    acc[half * U : (half + 1) * U, :],
                    wc[:, 0:U],
                    Xc[0:cc, b * N : (b + 1) * N],
                    start=True,  # per-partition zero region: each half starts its own rows
                    stop=False,
                    tile_position=(0, half * U),
                    # sim's group tracker isn't partition-base-aware; half 1 would
                    # falsely collide with half 0's pending group
                    skip_group_check=True,
                )
            for mi, m in enumerate((2, 4)):
                for k in range(KCH):
                    nc.tensor.matmul(
                        acc,
                        ac[p][0][:, k // 2, k % 2, mi, :, :],
                        tm_sb["bf16"][:, m, k // 2, k % 2, :],
                        start=False,
                        stop=False,
                        skip_group_check=True,
                    )
            for mi, m in enumerate((1, 3)):
                for kp in range(2):
                    nc.tensor.matmul(
                        acc,
                        ac[p][1][:, kp, :, mi, :, :],
                        tm_sb["fp8"][:, m, kp],
                        start=False,
                        stop=(mi == 1 and kp == 1),
                        perf_mode=DR,
                        skip_group_check=True,
                    )
            # per-half tanh + blend: batch b's h lands without waiting for
            # the pair, releasing the next phase's stage-1 per batch
            for half in range(2):
                b = 2 * p + half
                bcols = slice(b * N, (b + 1) * N)
                nc.scalar.activation(
                    out=Ct[sl, bcols],
                    in_=acc[half * U : (half + 1) * U, :],
                    func=AF.Tanh, bias=bc[:, 0:1], scale=1.0 / TSCALE,
                )
                # h_new = u*h + (1-u)*c
                nc.vector.tensor_mul(
                    out=wct[sl, bcols], in0=Wu[sl, bcols], in1=Ct[sl, bcols]
                )
                nc.vector.tensor_add(
                    out=h_dst(bcols), in0=uh[sl, bcols], in1=wct[sl, bcols]
                )
            post(p)

        return gate_phase, cand_phase

    def l0_h_src(cols):
        return X0[0:U, cols]

    def l0_r_dst(cols):
        return X0c[0:U, cols]

    def l0_h_dst(cols):
        # write h0' straight into X1 so g1's stage-1 doesn't wait on a copy
        return X1[0:U, cols]

    def l0_post(p):
        # fan h0' out to the other consumers, off the g1 critical path:
        # X1c feeds c1's stage-1 (medium slack), X0 feeds the NEXT step's g0
        # (GpSimd measured 5x slower than DVE for these copies)
        pcols = slice(2 * p * N, 2 * (p + 1) * N)
        nc.scalar.copy(out=X1c[0:U, pcols], in_=X1[0:U, pcols])
        nc.vector.tensor_copy(out=X0[0:U, pcols], in_=X1[0:U, pcols])

    def l1_h_src(cols):
        return X1[U:C1, cols]

    def l1_r_dst(cols):
        return X1c[U:C1, cols]

    def l1_h_dst(cols):
        return X1[U:C1, cols]

    def l1_post(p):
        pass

    def proj_phase(p, feed_cand=False):
        # projection for pair p: out = h1 . pw + pb (row 0 of pp)
        ppp = ps2.tile([128, 2, 512], F32, tag="s2", name="pp")
        for q in (2 * p, 2 * p + 1):
            pp = ppp[:, q % 2, :]
            nc.tensor.matmul(
                pp[0:2, :],
                pw_sb[64:128, :],
                X1[U:C1, q * 512 : (q + 1) * 512],
                start=True,
                stop=True,
            )
        # X0 row first (gates the next step's g0 stage-1), X0c after (only
        # read by the later cand stage-1)
        for q in (2 * p, 2 * p + 1):
            nc.scalar.activation(
                out=X0[U:C0, q * 512 : (q + 1) * 512],
                in_=ppp[0:1, q % 2, :],
                func=AF.Identity,
                bias=pb_sb[:, 0:1],
                scale=1.0,
            )
        if feed_cand:
            # decoder feedback: write the cand-path x row directly too,
            # replacing a serial 1-partition [1, BI] copy on the DVE
            for q in (2 * p, 2 * p + 1):
                nc.scalar.activation(
                    out=X0c[U:C0, q * 512 : (q + 1) * 512],
                    in_=ppp[0:1, q % 2, :],
                    func=AF.Identity,
                    bias=pb_sb[:, 0:1],
                    scale=1.0,
                )

    def build_step(pfx, fold0=False):
        if fold0:
            # decoder t>=1: x = h1.pw (+pb==0) folded into the L0 gate
            # weights, so the gate reads [h0; h1] from X1 and never waits on
            # the projection chain
            xg0, cg0, wg0 = X1, C1, w_sb["dwg0f"]
        else:
            xg0, cg0, wg0 = X0, C0, w_sb[f"{pfx}wg0"]
        g0, c0 = cell_phases(
            0, xg0, X0c, cg0, C0,
            wg0, w_sb[f"{pfx}bg0"], w_sb[f"{pfx}wc0"],
            w_sb[f"{pfx}bc0"], l0_h_src, l0_r_dst, l0_h_dst, l0_post,
        )
        g1, c1 = cell_phases(
            1, X1, X1c, C1, C1,
            w_sb[f"{pfx}wg1"], w_sb[f"{pfx}bg1"], w_sb[f"{pfx}wc1"],
            w_sb[f"{pfx}bc1"], l1_h_src, l1_r_dst, l1_h_dst, l1_post,
        )
        return g0, c0, g1, c1

    def stage_x(t):
        def emit(dst):
            nc.sync.dma_start(out=dst[U:C0, :], in_=d_xenc[t : t + 1, :])
        return emit

    def zero_x(dst):
        nc.vector.memset(dst[U:C0, :], 0.0)

    # Software-pipelined emission with a 1-phase skew: the trailing cand
    # phase (and decoder projection) of step t interleaves with step t+1's
    # gate matmuls so the PE never drains at a step boundary.
    # steps: list of (pfx, x_hook or None, dec_t or None)
    steps = []
    for t in range(n_enc):
        if t == 0:
            hook = None  # x_0 staged before the loop
        else:
            hook = stage_x(t)
        steps.append(("e", hook, None))
    steps.append(("d", (lambda dst: zero_x(dst)), 0))
    for t in range(1, n_dec):
        steps.append(("d", None, t))

    stage_x(0)(X0)
    stage_x(0)(X0c)
    pending = None  # (c1, dec_t) of the previous step
    for pfx, x_hook, dec_t in steps:
        fold0 = fold and dec_t is not None and x_hook is None
        g0, c0, g1, c1 = build_step(pfx, fold0)
        # x for THIS step must land before this step's g0/c0 read it; the
        # hook writes row 64 only, after the previous step's readers.
        if x_hook is not None:
            x_hook(X0)
        if pending is not None:
            # both cand-L1 pairs first: pair-1 PE work covers pair-0's
            # tanh+blend latency, then proj (which needs the blended h1)
            pc1, pdec = pending
            pc1(0)
            pc1(1)
            if pdec is not None:
                feed = dec_t is not None and x_hook is None
                proj_phase(0, feed_cand=feed)
                proj_phase(1, feed_cand=feed)
                nc.sync.dma_start(out=d_out[pdec : pdec + 1, :], in_=X0[U:C0, :])
        g0(0)
        g0(1)
        if x_hook is not None:
            x_hook(X0c)
        c0(0); c0(1)
        g1(0); g1(1)
        pending = (c1, dec_t)

    pc1, pdec = pending
    pc1(0); pc1(1)
    proj_phase(0); proj_phase(1)
    nc.sync.dma_start(out=d_out[pdec : pdec + 1, :], in_=X0[U:C0, :])

    for pool in (ps2, ps1, ac_pool, ag_pool, gpool, work, consts):
        pool.release()


# --------------------------------------------------------------------------
# host-side packing
# --------------------------------------------------------------------------
def _prep_shared(inputs):
    bf = mybir.dt.np(BF16)
    f8 = mybir.dt.np(FP8)
    sup = np.asarray(inputs["supports"], np.float64)
    eye = np.eye(N, dtype=np.float64)
    tms = [
        eye,
        sup[0],
        2.0 * (sup[0] @ sup[0]) - eye,
        sup[1],
        2.0 * (sup[1] @ sup[1]) - eye,
    ]
    # T (and the identity W blocks) are pre-scaled by TSCALE so fp8 entries
    # land in e4m3's normal range; the PSUM->SBUF activation undoes it via
    # scale=1/TSCALE.  |T|max ~1.05 -> 134 < 448, safe.
    tmats = np.stack([t.T * TSCALE for t in tms]).astype(np.float32)
    tmats = tmats.reshape(NM * KCH * 128, 512)

    shared = {}
    for s2 in ("fp8", "bf16"):
        dt_ = f8 if s2 == "fp8" else bf
        shared[f"tm_{s2}"] = np.ascontiguousarray(tmats.astype(dt_))
    for pfx, name in (("e", "enc"), ("d", "dec")):
        for lyr, c_in in ((0, C0), (1, C1)):
            wg = np.asarray(inputs[f"{name}{lyr}_Wg"], np.float32).reshape(
                c_in, NM * 2 * U
            )
            wc = np.asarray(inputs[f"{name}{lyr}_Wc"], np.float32).reshape(
                c_in, NM * U
            )
            bg = np.asarray(inputs[f"{name}{lyr}_bg"], np.float32)
            bc = np.asarray(inputs[f"{name}{lyr}_bc"], np.float32)
            # scale the identity (m=0) block to match the TSCALE'd T terms
            # (copy: the reshaped views alias the caller's input arrays)
            wg = wg.copy()
            wc = wc.copy()
            wg[:, 0 : 2 * U] *= TSCALE
            wc[:, 0:U] *= TSCALE
            if lyr == 0:
                perm = np.r_[1:c_in, 0]  # rows [h..., x]
                wg = wg[perm]
                wc = wc[perm]
            else:
                # layer-1 gate layout is [u; r] (see cell_phases): swap the
                # r/u column halves inside each m block, and the bias halves
                wg = np.ascontiguousarray(
                    wg.reshape(c_in, NM, 2, U)[:, :, ::-1, :].reshape(c_in, NM * 2 * U)
                )
                bg = np.concatenate([bg[U:], bg[:U]])
            shared[f"{pfx}wg{lyr}"] = np.ascontiguousarray(wg.astype(bf))
            if pfx == "d" and lyr == 0:
                pw_f = np.asarray(inputs["proj_W"], np.float64).reshape(U, 1)
                fold = np.vstack([wg[0:U], pw_f @ wg[U : U + 1]]).astype(np.float32)
                shared["dwg0f"] = np.ascontiguousarray(fold.astype(bf))
            shared[f"{pfx}wc{lyr}"] = np.ascontiguousarray(wc.astype(bf))
            shared[f"{pfx}bg{lyr}"] = np.ascontiguousarray(bg.reshape(2 * U, 1))
            shared[f"{pfx}bc{lyr}"] = np.ascontiguousarray(bc.reshape(U, 1))
    pw = np.asarray(inputs["proj_W"], np.float32).reshape(U, 1)
    shared["pw"] = np.ascontiguousarray(
        np.concatenate([pw, np.zeros((U, 1), np.float32)], axis=1).astype(bf)
    )
    shared["pb"] = np.asarray(inputs["proj_b"], np.float32).reshape(1, 1)
    return shared


def _make_in_maps(inputs, n_enc=T_ENC):
    bf = mybir.dt.np(BF16)
    shared = _prep_shared(inputs)
    x = np.asarray(inputs["inputs"], np.float32)  # (T, B, N)
    in_maps = []
    for c in range(NCORES):
        m = dict(shared)
        m["xenc"] = np.ascontiguousarray(
            x[:n_enc, c * BL : (c + 1) * BL, :].reshape(n_enc, BI).astype(bf)
        )
        in_maps.append(m)
    return in_maps


_PROG_CACHE = {}


def _get_program(n_enc=T_ENC, n_dec=HOR, fold=True):
    key = (n_enc, n_dec, fold)
    if key not in _PROG_CACHE:
        _PROG_CACHE[key] = _build_program(n_enc, n_dec, fold)
    return _PROG_CACHE[key]


def _run(inputs, n_enc=T_ENC, n_dec=HOR, **kw):
    fold = bool(np.allclose(np.asarray(inputs["proj_b"], np.float64), 0.0))
    nc = _get_program(n_enc, n_dec, fold)
    in_maps = _make_in_maps(inputs, n_enc)
    if not fold:
        for m in in_maps:
            m.pop("dwg0f", None)
    res = bass_utils.run_bass_kernel_spmd(nc, in_maps, core_ids=list(range(NCORES)), **kw)
    out = np.empty((n_dec, B, N), np.float32)
    for c in range(NCORES):
        out[:, c * BL : (c + 1) * BL, :] = (
            res.results[c]["outs"].astype(np.float32).reshape(n_dec, BL, N)
        )
    return out.reshape(n_dec, B, N), res


def kernel(**inputs) -> np.ndarray:
    out, _ = _run(inputs)
    return out.reshape(HOR, B, N)



# revision 34
# speedup vs baseline: 1.1102x; 1.0373x over previous
"""DCRNN (2-layer encoder/decoder DCGRU, N=512 nodes, B=32, U=64, K=2, 2 supports)
Trainium2 Bass/Tile kernel, data-parallel over batch across 8 NeuronCores.

Formulation: gconv(X) = sum_m T_m @ X @ W_m with T_m precomputed on host
(m=0 is the identity and is folded into stage 2 as a direct X @ W_0 matmul).
  stage 1 (dense):     A_m = X @ W_m, m=1..4  (X-chunk as lhsT -> node-major A)
  stage 2 (diffusion): out = X @ W_0 + sum_m (T_m A_m)^T, accumulated in PSUM,
                       bias+sigmoid/tanh fused into the PSUM->SBUF activation.
All matmul operands bf16 (or fp8e4m3 with DoubleRow for the diffusion stage:
two 128-row node chunks contracted per matmul at 2x rate). State tiles bf16.
Layout avoids every partition-shift copy:
  X0  [65,BI]  rows 0:64 h0,    row 64 x      (L0 gate lhsT)
  X0c [65,BI]  rows 0:64 r0*h0, row 64 x      (L0 cand lhsT)
  X1  [128,BI] rows 0:64 h0',   rows 64:128 h1        (L1 gate lhsT)
  X1c [128,BI] rows 0:64 h0',   rows 64:128 r1*h1     (L1 cand lhsT)
h1 and r1*h1 are written at partition base 64 directly by DVE (cross-base ok).
"""

import sys

sys.path.insert(0, "/opt/trn_rl_repo")

import numpy as np

import concourse.bass as bass
import concourse.mybir as mybir
import concourse.tile as tile
from concourse import bacc, bass_utils

# Model dims (fixed by the problem)
N = 512
T_ENC = 12
HOR = 12
U = 64
NM = 5  # diffusion matrices (I + 2 per support * 2 supports)
B = 32
NCORES = 8
BL = B // NCORES  # local batch = 4
BI = BL * N  # 2048: the (b, node) free dim
C0 = 1 + U  # 65 input channels, layer 0
C1 = U + U  # 128 input channels, layer 1
KCH = N // 128  # 4 node chunks

F32 = mybir.dt.float32
BF16 = mybir.dt.bfloat16
FP8 = mybir.dt.float8e4
AF = mybir.ActivationFunctionType
DR = mybir.MatmulPerfMode.DoubleRow
TSCALE = 128.0  # pre-scale on T & identity-W so fp8 T entries are normal-range

# stage-2 diffusion: gate = all-m fp8 DoubleRow; cand = m2,m4 bf16 (dominant
# terms) + m1,m3 fp8 DoubleRow (terms ~10% of output norm, fp8 error diluted)
import os as _os

LDW_OPT = _os.environ.get("LDW_OPT", "0") == "1"

_ldw_patched = False


def _patch_ldw_opt():
    global _ldw_patched
    if _ldw_patched or not LDW_OPT:
        return
    _ldw_patched = True
    orig = bass_utils.bir_verify_and_optimise

    def patched(tmpdir, inp="bir.json", outp="file.neff", arch=None, *, dve_root=None):
        import concourse.bass_utils as bu

        real_run = bu.run_command

        def run_hook(cmd, **kw):
            cmd = [
                c.replace("--enable-ldw-opt=false", "--enable-ldw-opt=true")
                for c in cmd
            ]
            return real_run(cmd, **kw)

        bu.run_command = run_hook
        try:
            return orig(tmpdir, inp, outp, arch, dve_root=dve_root)
        finally:
            bu.run_command = real_run

    bass_utils.bir_verify_and_optimise = patched


def _build_program(n_enc=T_ENC, n_dec=HOR, fold=True):
    _patch_ldw_opt()
    nc = bacc.Bacc("TRN2", target_bir_lowering=False, debug=False)

    # ---- DRAM I/O ----
    d_xenc = nc.dram_tensor("xenc", [n_enc, BI], BF16, kind="ExternalInput")
    d_tm = {}
    for s2 in ("fp8", "bf16"):
        dt_ = FP8 if s2 == "fp8" else BF16
        d_tm[s2] = nc.dram_tensor(
            f"tm_{s2}", [NM * KCH * 128, 512], dt_, kind="ExternalInput"
        )
    d_w = {}
    for pfx in ("e", "d"):
        for lyr, c_in in ((0, C0), (1, C1)):
            d_w[f"{pfx}wg{lyr}"] = nc.dram_tensor(
                f"{pfx}wg{lyr}", [c_in, NM * 2 * U], BF16, kind="ExternalInput"
            )
            d_w[f"{pfx}wc{lyr}"] = nc.dram_tensor(
                f"{pfx}wc{lyr}", [c_in, NM * U], BF16, kind="ExternalInput"
            )
            d_w[f"{pfx}bg{lyr}"] = nc.dram_tensor(
                f"{pfx}bg{lyr}", [2 * U, 1], F32, kind="ExternalInput"
            )
            d_w[f"{pfx}bc{lyr}"] = nc.dram_tensor(
                f"{pfx}bc{lyr}", [U, 1], F32, kind="ExternalInput"
            )
    if fold:
        d_w["dwg0f"] = nc.dram_tensor(
            "dwg0f", [C1, NM * 2 * U], BF16, kind="ExternalInput"
        )
    d_pw = nc.dram_tensor("pw", [U, 2], BF16, kind="ExternalInput")
    d_pb = nc.dram_tensor("pb", [1, 1], F32, kind="ExternalInput")
    d_out = nc.dram_tensor("outs", [n_dec, BI], BF16, kind="ExternalOutput")

    with tile.TileContext(nc) as tc:
        _body(tc, n_enc, n_dec, d_xenc, d_tm, d_w, d_pw, d_pb, d_out, fold)
    nc.compile()
    return nc


def _body(tc, n_enc, n_dec, d_xenc, d_tm, d_w, d_pw, d_pb, d_out, fold):
    nc = tc.nc
    consts = tc.alloc_tile_pool(name="consts", bufs=1)
    work = tc.alloc_tile_pool(name="work", bufs=1)
    gpool = tc.alloc_tile_pool(name="gpool", bufs=2)
    ag_pool = tc.alloc_tile_pool(name="agp", bufs=10)
    ac_pool = tc.alloc_tile_pool(name="acp", bufs=6)
    ps1 = tc.alloc_tile_pool(name="ps1", bufs=2, space="PSUM")
    ps2 = tc.alloc_tile_pool(name="ps2", bufs=2, space="PSUM")

    # ---- resident constants ----
    # tm layout: [128(p), m, kpair, j, 512]; node index = (kpair*2+j)*128 + p
    tm_sb = {}
    for s2 in ("fp8", "bf16"):
        dt_ = FP8 if s2 == "fp8" else BF16
        t = consts.tile([128, NM, 2, 2, 512], dt_, name=f"tm_sb_{s2}")
        tm_sb[s2] = t
        for m in range(NM):
            for k in range(KCH):
                row = (m * KCH + k) * 128
                nc.sync.dma_start(
                    out=t[:, m, k // 2, k % 2, :], in_=d_tm[s2][row : row + 128, :]
                )

    w_sb = {}
    for key, dt_ in d_w.items():
        shape = list(dt_.shape)
        sb_dt = BF16 if key[1] == "w" else F32
        w_sb[key] = consts.tile(shape, sb_dt, name=f"sb_{key}")
        nc.sync.dma_start(out=w_sb[key][:, :], in_=dt_[:, :])
    pw_sb = consts.tile([128, 2], BF16, name="pw_sb")
    nc.sync.dma_start(out=pw_sb[64:128, :], in_=d_pw[:, :])
    pb_sb = consts.tile([1, 1], F32, name="pb_sb")
    nc.sync.dma_start(out=pb_sb, in_=d_pb[:, :])

    # ---- persistent state ----
    X0 = work.tile([C0, BI], BF16, name="X0")  # [h0 ; x]
    X0c = work.tile([C0, BI], BF16, name="X0c")  # [r0*h0 ; x]
    X1 = work.tile([C1, BI], BF16, name="X1")  # [h0 ; h1]
    X1c = work.tile([C1, BI], BF16, name="X1c")  # [h0 ; r1*h1]

    nc.gpsimd.memset(X0[0:U, :], 0.0)
    nc.gpsimd.memset(X0c[0:U, :], 0.0)
    nc.gpsimd.memset(X1[:, :], 0.0)
    nc.gpsimd.memset(X1c[:, :], 0.0)

    tc.strict_bb_all_engine_barrier()

    def cell_phases(lyr, Xg, Xc, cg, cc, wg, bg, wc, bc, h_src, r_dst, h_dst, post):
        """One DCGRU cell, split into per-batch-pair phases.

        Layer-l elementwise state lives at partition base l*64 so every
        two-tensor DVE op has matching input bases (h1 sits at rows 64:128 of
        X1).  The gate output layout is [r; u] for layer 0 and [u; r] for
        layer 1 (weights pre-flipped on host), so r shares a base with h; the
        u half is moved across with one single-src copy per pair.
        """
        gdt = FP8
        sl = slice(lyr * U, (lyr + 1) * U)  # this layer's partition rows
        u_src = slice(U, 2 * U) if lyr == 0 else slice(0, U)  # u half of RU
        r_src = slice(0, U) if lyr == 0 else slice(U, 2 * U)  # r half of RU
        RU = gpool.tile([2 * U, BI], BF16, tag="RU", name="RU", bufs=2)
        Uu = gpool.tile([2 * U, BI], BF16, tag="Uu", name="Uu", bufs=2)
        Wu = gpool.tile([2 * U, BI], BF16, tag="Wu", name="Wu", bufs=2)
        uh = gpool.tile([2 * U, BI], BF16, tag="uh", name="uh", bufs=2)
        Ct = gpool.tile([2 * U, BI], BF16, tag="Ct", name="Ct", bufs=2)
        wct = gpool.tile([2 * U, BI], BF16, tag="wct", name="wct", bufs=2)
        ag = {}
        ac = {}
        cacc = [None]

        def gate_s1(p):
            # stage 1: A_m = X @ Wg_m for m=1..4 (m=0 folded into stage 2).
            # Two node chunks share a 2-bank PSUM pair tile -> one wide copy.
            for b in (2 * p, 2 * p + 1):
                for kp in range(2):
                    pg = ps1.tile([128, 2, 512], F32, tag="s1g", name="pg")
                    for j in range(2):
                        k = kp * 2 + j
                        lhsT = Xg[0:cg, b * N + k * 128 : b * N + (k + 1) * 128]
                        nc.tensor.matmul(
                            pg[:, j, :], lhsT, wg[:, 128:640], start=True, stop=True
                        )
                    a = ag_pool.tile(
                        [128, 2, NM - 1, 128], gdt, tag="ag", name="ag"
                    )
                    ag[(b, kp)] = a
                    # split the PSUM->SBUF A copies across Scalar and Vector
                    # so neither queue rate-limits the gate phase
                    srcv = pg.rearrange("p j (m c) -> p j m c", m=NM - 1)
                    if (b + kp) % 2 == 0:
                        nc.scalar.copy(out=a[:, :, :, :], in_=srcv)
                    else:
                        nc.vector.tensor_copy(out=a[:, :, :, :], in_=srcv)

        def gate_s2(p):
            # stage 2: acc = X @ Wg_0 + sum_{m>0} (T_m A_m)^T, fused sigmoid.
            # The two batches of the pair share a 2-bank acc -> one wide act.
            accp = ps2.tile([128, 2, 512], F32, tag="s2", name="accg")
            for half, b in enumerate((2 * p, 2 * p + 1)):
                acc = accp[:, half, :]
                nc.tensor.matmul(
                    acc,
                    wg[:, 0:128],
                    Xg[0:cg, b * N : (b + 1) * N],
                    start=True,
                    stop=False,
                )
                for m in range(1, NM):
                    for kp in range(2):
                        nc.tensor.matmul(
                            acc,
                            ag[(b, kp)][:, :, m - 1, :],
                            tm_sb["fp8"][:, m, kp],
                            start=False,
                            stop=(m == NM - 1 and kp == 1),
                            perf_mode=DR,
                        )
            # per-half acts + DVE chain: each batch's sigmoid -> r*h releases
            # that batch's cand stage-1 without waiting for the pair
            for half, b in enumerate((2 * p, 2 * p + 1)):
                bcols = slice(b * N, (b + 1) * N)
                nc.scalar.activation(
                    out=RU[:, bcols], in_=accp[:, half, :],
                    func=AF.Sigmoid, bias=bg[:, 0:1], scale=1.0 / TSCALE,
                )
                # r*h first: it gates the cand phase's stage-1 (critical path)
                nc.vector.tensor_mul(
                    out=r_dst(bcols), in0=RU[r_src, bcols], in1=h_src(bcols)
                )
            pcols = slice(2 * p * N, 2 * (p + 1) * N)
            # move u to this layer's partition rows (single-src cross-base copy)
            nc.vector.tensor_copy(out=Uu[sl, pcols], in_=RU[u_src, pcols])
            nc.vector.tensor_mul(
                out=uh[sl, pcols], in0=Uu[sl, pcols], in1=h_src(pcols)
            )
            nc.vector.tensor_scalar(
                out=Wu[sl, pcols], in0=Uu[sl, pcols],
                scalar1=-1.0, scalar2=1.0,
                op0=mybir.AluOpType.mult, op1=mybir.AluOpType.add,
            )

        def cand_s1(p):
            # stage 1: m=1..4 only; all 4 chunks of one batch share a 2-bank
            # pair tile (256 cols each) -> one wide copy per batch
            for half, b in enumerate((2 * p, 2 * p + 1)):
                pc = ps1.tile([128, 2, 512], F32, tag="s1g", name="pc")
                for k in range(KCH):
                    lhsT = Xc[0:cc, b * N + k * 128 : b * N + (k + 1) * 128]
                    nc.tensor.matmul(
                        pc[:, k // 2, (k % 2) * 256 : (k % 2) * 256 + 256],
                        lhsT,
                        wc[:, U : NM * U],
                        start=True,
                        stop=True,
                        skip_group_check=(k % 2 == 1),
                    )
                if half == 0:
                    # split A by m: m2,m4 -> bf16 (dominant), m1,m3 -> fp8 DR
                    ac[p] = (
                        ac_pool.tile([128, 2, 2, 2, 2, U], BF16, tag="a24", name="a24"),
                        ac_pool.tile([128, 2, 2, 2, 2, U], FP8, tag="a13", name="a13"),
                    )
                # pc free layout (kp, j, m, u); m = 2*m2 + mm: mm=0 -> m13,
                # mm=1 -> m24 (m index here is m-1 for m=1..4)
                srcv = pc.rearrange(
                    "p kp (j m2 mm u) -> p kp j m2 mm u", j=2, m2=2, mm=2
                )
                # balance the two casts across Scalar and Vector per half
                e24 = nc.vector if half == 0 else nc.scalar
                e13 = nc.scalar if half == 0 else nc.vector
                (e24.tensor_copy if e24 is nc.vector else e24.copy)(
                    out=ac[p][0][:, :, :, :, half, :], in_=srcv[:, :, :, :, 1, :]
                )
                (e13.tensor_copy if e13 is nc.vector else e13.copy)(
                    out=ac[p][1][:, :, :, :, half, :], in_=srcv[:, :, :, :, 0, :]
                )

        def cand_s2(p):
            # stage 2: identity fold (col-tiled pair) + diffusion, fused tanh
            # (both p's share one 2-bank pair tile, one bank each)
            if cacc[0] is None:
                cacc[0] = ps2.tile([128, 2, 512], F32, tag="s2", name="accc")
            acc = cacc[0][:, p, :]
            for half in range(2):
                b = 2 * p + half
                nc.tensor.matmul(
                    acc[half * U : (half + 1) * U, :],
                    wc[:, 0:U],
                    Xc[0:cc, b * N : (b + 1) * N],
                    start=True,  # per-partition zero region: each half starts its own rows
                    stop=False,
                    tile_position=(0, half * U),
                    # sim's group tracker isn't partition-base-aware; half 1 would
                    # falsely collide with half 0's pending group
                    skip_group_check=True,
                )
            for mi, m in enumerate((2, 4)):
                for k in range(KCH):
                    nc.tensor.matmul(
                        acc,
                        ac[p][0][:, k // 2, k % 2, mi, :, :],
                        tm_sb["bf16"][:, m, k // 2, k % 2, :],
                        start=False,
                        stop=False,
                        skip_group_check=True,
                    )
            for mi, m in enumerate((1, 3)):
                for kp in range(2):
                    nc.tensor.matmul(
                        acc,
                        ac[p][1][:, kp, :, mi, :, :],
                        tm_sb["fp8"][:, m, kp],
                        start=False,
                        stop=(mi == 1 and kp == 1),
                        perf_mode=DR,
                        skip_group_check=True,
                    )
            # per-half tanh + blend: batch b's h lands without waiting for
            # the pair, releasing the next phase's stage-1 per batch
            for half in range(2):
                b = 2 * p + half
                bcols = slice(b * N, (b + 1) * N)
                nc.scalar.activation(
                    out=Ct[sl, bcols],
                    in_=acc[half * U : (half + 1) * U, :],
                    func=AF.Tanh, bias=bc[:, 0:1], scale=1.0 / TSCALE,
                )
                # h_new = u*h + (1-u)*c
                nc.vector.tensor_mul(
                    out=wct[sl, bcols], in0=Wu[sl, bcols], in1=Ct[sl, bcols]
                )
                nc.vector.tensor_add(
                    out=h_dst(bcols), in0=uh[sl, bcols], in1=wct[sl, bcols]
                )
            post(p)

        return gate_s1, gate_s2, cand_s1, cand_s2

    def l0_h_src(cols):
        return X0[0:U, cols]

    def l0_r_dst(cols):
        return X0c[0:U, cols]

    def l0_h_dst(cols):
        # write h0' straight into X1 so g1's stage-1 doesn't wait on a copy
        return X1[0:U, cols]

    def l0_post(p):
        # fan h0' out to the other consumers, off the g1 critical path:
        # X1c feeds c1's stage-1 (medium slack), X0 feeds the NEXT step's g0
        # (GpSimd measured 5x slower than DVE for these copies)
        pcols = slice(2 * p * N, 2 * (p + 1) * N)
        nc.scalar.copy(out=X1c[0:U, pcols], in_=X1[0:U, pcols])
        nc.vector.tensor_copy(out=X0[0:U, pcols], in_=X1[0:U, pcols])

    def l1_h_src(cols):
        return X1[U:C1, cols]

    def l1_r_dst(cols):
        return X1c[U:C1, cols]

    def l1_h_dst(cols):
        return X1[U:C1, cols]

    def l1_post(p):
        pass

    def proj_phase(p, feed_cand=False):
        # projection for pair p: out = h1 . pw + pb (row 0 of pp)
        ppp = ps2.tile([128, 2, 512], F32, tag="s2", name="pp")
        for q in (2 * p, 2 * p + 1):
            pp = ppp[:, q % 2, :]
            nc.tensor.matmul(
                pp[0:2, :],
                pw_sb[64:128, :],
                X1[U:C1, q * 512 : (q + 1) * 512],
                start=True,
                stop=True,
            )
        # X0 row first (gates the next step's g0 stage-1), X0c after (only
        # read by the later cand stage-1)
        for q in (2 * p, 2 * p + 1):
            nc.scalar.activation(
                out=X0[U:C0, q * 512 : (q + 1) * 512],
                in_=ppp[0:1, q % 2, :],
                func=AF.Identity,
                bias=pb_sb[:, 0:1],
                scale=1.0,
            )
        if feed_cand:
            # decoder feedback: write the cand-path x row directly too,
            # replacing a serial 1-partition [1, BI] copy on the DVE
            for q in (2 * p, 2 * p + 1):
                nc.scalar.activation(
                    out=X0c[U:C0, q * 512 : (q + 1) * 512],
                    in_=ppp[0:1, q % 2, :],
                    func=AF.Identity,
                    bias=pb_sb[:, 0:1],
                    scale=1.0,
                )

    def build_step(pfx, fold0=False):
        if fold0:
            # decoder t>=1: x = h1.pw (+pb==0) folded into the L0 gate
            # weights, so the gate reads [h0; h1] from X1 and never waits on
            # the projection chain
            xg0, cg0, wg0 = X1, C1, w_sb["dwg0f"]
        else:
            xg0, cg0, wg0 = X0, C0, w_sb[f"{pfx}wg0"]
        c0_cell = cell_phases(
            0, xg0, X0c, cg0, C0,
            wg0, w_sb[f"{pfx}bg0"], w_sb[f"{pfx}wc0"],
            w_sb[f"{pfx}bc0"], l0_h_src, l0_r_dst, l0_h_dst, l0_post,
        )
        c1_cell = cell_phases(
            1, X1, X1c, C1, C1,
            w_sb[f"{pfx}wg1"], w_sb[f"{pfx}bg1"], w_sb[f"{pfx}wc1"],
            w_sb[f"{pfx}bc1"], l1_h_src, l1_r_dst, l1_h_dst, l1_post,
        )
        return c0_cell, c1_cell

    def stage_x(t):
        def emit(dst):
            nc.sync.dma_start(out=dst[U:C0, :], in_=d_xenc[t : t + 1, :])
        return emit

    def zero_x(dst):
        nc.vector.memset(dst[U:C0, :], 0.0)

    # Software-pipelined emission with a 1-phase skew: the trailing cand
    # phase (and decoder projection) of step t interleaves with step t+1's
    # gate matmuls so the PE never drains at a step boundary.
    # steps: list of (pfx, x_hook or None, dec_t or None)
    steps = []
    for t in range(n_enc):
        if t == 0:
            hook = None  # x_0 staged before the loop
        else:
            hook = stage_x(t)
        steps.append(("e", hook, None))
    steps.append(("d", (lambda dst: zero_x(dst)), 0))
    for t in range(1, n_dec):
        steps.append(("d", None, t))

    stage_x(0)(X0)
    stage_x(0)(X0c)
    pending = None  # (c1, dec_t) of the previous step
    # Sub-phase schedule: s1(0), s1(1), s2(0), s2(1) per cell so pair-1's
    # stage-1 PE work always covers pair-0's PSUM->SBUF cast latency, and the
    # trailing cand-L1 stage-2 of step t runs at the head of step t+1.
    for pfx, x_hook, dec_t in steps:
        fold0 = fold and dec_t is not None and x_hook is None
        (g0s1, g0s2, c0s1, c0s2), (g1s1, g1s2, c1s1, c1s2) = build_step(
            pfx, fold0
        )
        # x for THIS step must land before this step's g0/c0 read it; the
        # hook writes row 64 only, after the previous step's readers.
        if x_hook is not None:
            x_hook(X0)
        if pending is not None:
            pc1s2, pdec = pending
            pc1s2(0); pc1s2(1)
            if pdec is not None:
                feed = dec_t is not None and x_hook is None
                proj_phase(0, feed_cand=feed)
                proj_phase(1, feed_cand=feed)
                nc.sync.dma_start(out=d_out[pdec : pdec + 1, :], in_=X0[U:C0, :])
        g0s1(0); g0s1(1)
        g0s2(0); g0s2(1)
        if x_hook is not None:
            x_hook(X0c)
        c0s1(0); c0s1(1)
        c0s2(0); c0s2(1)
        g1s1(0); g1s1(1)
        g1s2(0); g1s2(1)
        c1s1(0); c1s1(1)
        pending = (c1s2, dec_t)

    pc1s2, pdec = pending
    pc1s2(0); pc1s2(1)
    proj_phase(0); proj_phase(1)
    nc.sync.dma_start(out=d_out[pdec : pdec + 1, :], in_=X0[U:C0, :])

    for pool in (ps2, ps1, ac_pool, ag_pool, gpool, work, consts):
        pool.release()


# --------------------------------------------------------------------------
# host-side packing
# --------------------------------------------------------------------------
def _prep_shared(inputs):
    bf = mybir.dt.np(BF16)
    f8 = mybir.dt.np(FP8)
    sup = np.asarray(inputs["supports"], np.float64)
    eye = np.eye(N, dtype=np.float64)
    tms = [
        eye,
        sup[0],
        2.0 * (sup[0] @ sup[0]) - eye,
        sup[1],
        2.0 * (sup[1] @ sup[1]) - eye,
    ]
    # T (and the identity W blocks) are pre-scaled by TSCALE so fp8 entries
    # land in e4m3's normal range; the PSUM->SBUF activation undoes it via
    # scale=1/TSCALE.  |T|max ~1.05 -> 134 < 448, safe.
    tmats = np.stack([t.T * TSCALE for t in tms]).astype(np.float32)
    tmats = tmats.reshape(NM * KCH * 128, 512)

    shared = {}
    for s2 in ("fp8", "bf16"):
        dt_ = f8 if s2 == "fp8" else bf
        shared[f"tm_{s2}"] = np.ascontiguousarray(tmats.astype(dt_))
    for pfx, name in (("e", "enc"), ("d", "dec")):
        for lyr, c_in in ((0, C0), (1, C1)):
            wg = np.asarray(inputs[f"{name}{lyr}_Wg"], np.float32).reshape(
                c_in, NM * 2 * U
            )
            wc = np.asarray(inputs[f"{name}{lyr}_Wc"], np.float32).reshape(
                c_in, NM * U
            )
            bg = np.asarray(inputs[f"{name}{lyr}_bg"], np.float32)
            bc = np.asarray(inputs[f"{name}{lyr}_bc"], np.float32)
            # scale the identity (m=0) block to match the TSCALE'd T terms
            # (copy: the reshaped views alias the caller's input arrays)
            wg = wg.copy()
            wc = wc.copy()
            wg[:, 0 : 2 * U] *= TSCALE
            wc[:, 0:U] *= TSCALE
            if lyr == 0:
                perm = np.r_[1:c_in, 0]  # rows [h..., x]
                wg = wg[perm]
                wc = wc[perm]
            else:
                # layer-1 gate layout is [u; r] (see cell_phases): swap the
                # r/u column halves inside each m block, and the bias halves
                wg = np.ascontiguousarray(
                    wg.reshape(c_in, NM, 2, U)[:, :, ::-1, :].reshape(c_in, NM * 2 * U)
                )
                bg = np.concatenate([bg[U:], bg[:U]])
            shared[f"{pfx}wg{lyr}"] = np.ascontiguousarray(wg.astype(bf))
            if pfx == "d" and lyr == 0:
                pw_f = np.asarray(inputs["proj_W"], np.float64).reshape(U, 1)
                fold = np.vstack([wg[0:U], pw_f @ wg[U : U + 1]]).astype(np.float32)
                shared["dwg0f"] = np.ascontiguousarray(fold.astype(bf))
            shared[f"{pfx}wc{lyr}"] = np.ascontiguousarray(wc.astype(bf))
            shared[f"{pfx}bg{lyr}"] = np.ascontiguousarray(bg.reshape(2 * U, 1))
            shared[f"{pfx}bc{lyr}"] = np.ascontiguousarray(bc.reshape(U, 1))
    pw = np.asarray(inputs["proj_W"], np.float32).reshape(U, 1)
    shared["pw"] = np.ascontiguousarray(
        np.concatenate([pw, np.zeros((U, 1), np.float32)], axis=1).astype(bf)
    )
    shared["pb"] = np.asarray(inputs["proj_b"], np.float32).reshape(1, 1)
    return shared


def _make_in_maps(inputs, n_enc=T_ENC):
    bf = mybir.dt.np(BF16)
    shared = _prep_shared(inputs)
    x = np.asarray(inputs["inputs"], np.float32)  # (T, B, N)
    in_maps = []
    for c in range(NCORES):
        m = dict(shared)
        m["xenc"] = np.ascontiguousarray(
            x[:n_enc, c * BL : (c + 1) * BL, :].reshape(n_enc, BI).astype(bf)
        )
        in_maps.append(m)
    return in_maps


_PROG_CACHE = {}


def _get_program(n_enc=T_ENC, n_dec=HOR, fold=True):
    key = (n_enc, n_dec, fold)
    if key not in _PROG_CACHE:
        _PROG_CACHE[key] = _build_program(n_enc, n_dec, fold)
    return _PROG_CACHE[key]


def _run(inputs, n_enc=T_ENC, n_dec=HOR, **kw):
    fold = bool(np.allclose(np.asarray(inputs["proj_b"], np.float64), 0.0))
    nc = _get_program(n_enc, n_dec, fold)
    in_maps = _make_in_maps(inputs, n_enc)
    if not fold:
        for m in in_maps:
            m.pop("dwg0f", None)
    res = bass_utils.run_bass_kernel_spmd(nc, in_maps, core_ids=list(range(NCORES)), **kw)
    out = np.empty((n_dec, B, N), np.float32)
    for c in range(NCORES):
        out[:, c * BL : (c + 1) * BL, :] = (
            res.results[c]["outs"].astype(np.float32).reshape(n_dec, BL, N)
        )
    return out.reshape(n_dec, B, N), res


def kernel(**inputs) -> np.ndarray:
    out, _ = _run(inputs)
    return out.reshape(HOR, B, N)



# revision 35
# speedup vs baseline: 1.1153x; 1.0047x over previous
"""DCRNN (2-layer encoder/decoder DCGRU, N=512 nodes, B=32, U=64, K=2, 2 supports)
Trainium2 Bass/Tile kernel, data-parallel over batch across 8 NeuronCores.

Formulation: gconv(X) = sum_m T_m @ X @ W_m with T_m precomputed on host
(m=0 is the identity and is folded into stage 2 as a direct X @ W_0 matmul).
  stage 1 (dense):     A_m = X @ W_m, m=1..4  (X-chunk as lhsT -> node-major A)
  stage 2 (diffusion): out = X @ W_0 + sum_m (T_m A_m)^T, accumulated in PSUM,
                       bias+sigmoid/tanh fused into the PSUM->SBUF activation.
All matmul operands bf16 (or fp8e4m3 with DoubleRow for the diffusion stage:
two 128-row node chunks contracted per matmul at 2x rate). State tiles bf16.
Layout avoids every partition-shift copy:
  X0  [65,BI]  rows 0:64 h0,    row 64 x      (L0 gate lhsT)
  X0c [65,BI]  rows 0:64 r0*h0, row 64 x      (L0 cand lhsT)
  X1  [128,BI] rows 0:64 h0',   rows 64:128 h1        (L1 gate lhsT)
  X1c [128,BI] rows 0:64 h0',   rows 64:128 r1*h1     (L1 cand lhsT)
h1 and r1*h1 are written at partition base 64 directly by DVE (cross-base ok).
"""

import sys

sys.path.insert(0, "/opt/trn_rl_repo")

import numpy as np

import concourse.bass as bass
import concourse.mybir as mybir
import concourse.tile as tile
from concourse import bacc, bass_utils

# Model dims (fixed by the problem)
N = 512
T_ENC = 12
HOR = 12
U = 64
NM = 5  # diffusion matrices (I + 2 per support * 2 supports)
B = 32
NCORES = 8
BL = B // NCORES  # local batch = 4
BI = BL * N  # 2048: the (b, node) free dim
C0 = 1 + U  # 65 input channels, layer 0
C1 = U + U  # 128 input channels, layer 1
KCH = N // 128  # 4 node chunks

F32 = mybir.dt.float32
BF16 = mybir.dt.bfloat16
FP8 = mybir.dt.float8e4
AF = mybir.ActivationFunctionType
DR = mybir.MatmulPerfMode.DoubleRow
TSCALE = 128.0  # pre-scale on T & identity-W so fp8 T entries are normal-range

# stage-2 diffusion: gate = all-m fp8 DoubleRow; cand = m2,m4 bf16 (dominant
# terms) + m1,m3 fp8 DoubleRow (terms ~10% of output norm, fp8 error diluted)
import os as _os

LDW_OPT = _os.environ.get("LDW_OPT", "0") == "1"

_ldw_patched = False


def _patch_ldw_opt():
    global _ldw_patched
    if _ldw_patched or not LDW_OPT:
        return
    _ldw_patched = True
    orig = bass_utils.bir_verify_and_optimise

    def patched(tmpdir, inp="bir.json", outp="file.neff", arch=None, *, dve_root=None):
        import concourse.bass_utils as bu

        real_run = bu.run_command

        def run_hook(cmd, **kw):
            cmd = [
                c.replace("--enable-ldw-opt=false", "--enable-ldw-opt=true")
                for c in cmd
            ]
            return real_run(cmd, **kw)

        bu.run_command = run_hook
        try:
            return orig(tmpdir, inp, outp, arch, dve_root=dve_root)
        finally:
            bu.run_command = real_run

    bass_utils.bir_verify_and_optimise = patched


def _build_program(n_enc=T_ENC, n_dec=HOR, fold=True):
    _patch_ldw_opt()
    nc = bacc.Bacc("TRN2", target_bir_lowering=False, debug=False)

    # ---- DRAM I/O ----
    d_xenc = nc.dram_tensor("xenc", [n_enc, BI], BF16, kind="ExternalInput")
    d_tm = {}
    for s2 in ("fp8", "bf16"):
        dt_ = FP8 if s2 == "fp8" else BF16
        d_tm[s2] = nc.dram_tensor(
            f"tm_{s2}", [NM * KCH * 128, 512], dt_, kind="ExternalInput"
        )
    d_w = {}
    for pfx in ("e", "d"):
        for lyr, c_in in ((0, C0), (1, C1)):
            d_w[f"{pfx}wg{lyr}"] = nc.dram_tensor(
                f"{pfx}wg{lyr}", [c_in, NM * 2 * U], BF16, kind="ExternalInput"
            )
            d_w[f"{pfx}wc{lyr}"] = nc.dram_tensor(
                f"{pfx}wc{lyr}", [c_in, NM * U], BF16, kind="ExternalInput"
            )
            d_w[f"{pfx}bg{lyr}"] = nc.dram_tensor(
                f"{pfx}bg{lyr}", [2 * U, 1], F32, kind="ExternalInput"
            )
            d_w[f"{pfx}bc{lyr}"] = nc.dram_tensor(
                f"{pfx}bc{lyr}", [U, 1], F32, kind="ExternalInput"
            )
    if fold:
        d_w["dwg0f"] = nc.dram_tensor(
            "dwg0f", [C1, NM * 2 * U], BF16, kind="ExternalInput"
        )
    d_pw = nc.dram_tensor("pw", [U, 2], BF16, kind="ExternalInput")
    d_pb = nc.dram_tensor("pb", [1, 1], F32, kind="ExternalInput")
    d_out = nc.dram_tensor("outs", [n_dec, BI], BF16, kind="ExternalOutput")

    with tile.TileContext(nc) as tc:
        _body(tc, n_enc, n_dec, d_xenc, d_tm, d_w, d_pw, d_pb, d_out, fold)
    nc.compile()
    return nc


def _body(tc, n_enc, n_dec, d_xenc, d_tm, d_w, d_pw, d_pb, d_out, fold):
    nc = tc.nc
    consts = tc.alloc_tile_pool(name="consts", bufs=1)
    work = tc.alloc_tile_pool(name="work", bufs=1)
    gpool = tc.alloc_tile_pool(name="gpool", bufs=2)
    ag_pool = tc.alloc_tile_pool(name="agp", bufs=10)
    ac_pool = tc.alloc_tile_pool(name="acp", bufs=6)
    ps1 = tc.alloc_tile_pool(name="ps1", bufs=2, space="PSUM")
    ps2 = tc.alloc_tile_pool(name="ps2", bufs=4, space="PSUM")

    # ---- resident constants ----
    # tm layout: [128(p), m, kpair, j, 512]; node index = (kpair*2+j)*128 + p
    tm_sb = {}
    for s2 in ("fp8", "bf16"):
        dt_ = FP8 if s2 == "fp8" else BF16
        t = consts.tile([128, NM, 2, 2, 512], dt_, name=f"tm_sb_{s2}")
        tm_sb[s2] = t
        for m in range(NM):
            for k in range(KCH):
                row = (m * KCH + k) * 128
                nc.sync.dma_start(
                    out=t[:, m, k // 2, k % 2, :], in_=d_tm[s2][row : row + 128, :]
                )

    w_sb = {}
    for key, dt_ in d_w.items():
        shape = list(dt_.shape)
        sb_dt = BF16 if key[1] == "w" else F32
        w_sb[key] = consts.tile(shape, sb_dt, name=f"sb_{key}")
        nc.sync.dma_start(out=w_sb[key][:, :], in_=dt_[:, :])
    pw_sb = consts.tile([128, 2], BF16, name="pw_sb")
    nc.sync.dma_start(out=pw_sb[64:128, :], in_=d_pw[:, :])
    pb_sb = consts.tile([1, 1], F32, name="pb_sb")
    nc.sync.dma_start(out=pb_sb, in_=d_pb[:, :])

    # ---- persistent state ----
    X0 = work.tile([C0, BI], BF16, name="X0")  # [h0 ; x]
    X0c = work.tile([C0, BI], BF16, name="X0c")  # [r0*h0 ; x]
    X1 = work.tile([C1, BI], BF16, name="X1")  # [h0 ; h1]
    X1c = work.tile([C1, BI], BF16, name="X1c")  # [h0 ; r1*h1]

    nc.gpsimd.memset(X0[0:U, :], 0.0)
    nc.gpsimd.memset(X0c[0:U, :], 0.0)
    nc.gpsimd.memset(X1[:, :], 0.0)
    nc.gpsimd.memset(X1c[:, :], 0.0)

    tc.strict_bb_all_engine_barrier()

    def cell_phases(lyr, Xg, Xc, cg, cc, wg, bg, wc, bc, h_src, r_dst, h_dst, post):
        """One DCGRU cell, split into per-batch-pair phases.

        Layer-l elementwise state lives at partition base l*64 so every
        two-tensor DVE op has matching input bases (h1 sits at rows 64:128 of
        X1).  The gate output layout is [r; u] for layer 0 and [u; r] for
        layer 1 (weights pre-flipped on host), so r shares a base with h; the
        u half is moved across with one single-src copy per pair.
        """
        gdt = FP8
        sl = slice(lyr * U, (lyr + 1) * U)  # this layer's partition rows
        u_src = slice(U, 2 * U) if lyr == 0 else slice(0, U)  # u half of RU
        r_src = slice(0, U) if lyr == 0 else slice(U, 2 * U)  # r half of RU
        RU = gpool.tile([2 * U, BI], BF16, tag="RU", name="RU", bufs=2)
        Uu = gpool.tile([2 * U, BI], BF16, tag="Uu", name="Uu", bufs=2)
        Wu = gpool.tile([2 * U, BI], BF16, tag="Wu", name="Wu", bufs=2)
        uh = gpool.tile([2 * U, BI], BF16, tag="uh", name="uh", bufs=2)
        Ct = gpool.tile([2 * U, BI], BF16, tag="Ct", name="Ct", bufs=2)
        wct = gpool.tile([2 * U, BI], BF16, tag="wct", name="wct", bufs=2)
        ag = {}
        ac = {}

        def gate_s1(p):
            # stage 1: A_m = X @ Wg_m for m=1..4 (m=0 folded into stage 2).
            # Two node chunks share a 2-bank PSUM pair tile -> one wide copy.
            for b in (2 * p, 2 * p + 1):
                for kp in range(2):
                    pg = ps1.tile([128, 2, 512], F32, tag="s1g", name="pg")
                    for j in range(2):
                        k = kp * 2 + j
                        lhsT = Xg[0:cg, b * N + k * 128 : b * N + (k + 1) * 128]
                        nc.tensor.matmul(
                            pg[:, j, :], lhsT, wg[:, 128:640], start=True, stop=True
                        )
                    a = ag_pool.tile(
                        [128, 2, NM - 1, 128], gdt, tag="ag", name="ag"
                    )
                    ag[(b, kp)] = a
                    # split the PSUM->SBUF A copies across Scalar and Vector
                    # so neither queue rate-limits the gate phase
                    srcv = pg.rearrange("p j (m c) -> p j m c", m=NM - 1)
                    if (b + kp) % 2 == 0:
                        nc.scalar.copy(out=a[:, :, :, :], in_=srcv)
                    else:
                        nc.vector.tensor_copy(out=a[:, :, :, :], in_=srcv)

        def gate_s2(p):
            # stage 2: acc = X @ Wg_0 + sum_{m>0} (T_m A_m)^T, fused sigmoid.
            # Per-half 1-bank acc tiles keep the ps2 ring loosely coupled.
            accs = {}
            for half, b in enumerate((2 * p, 2 * p + 1)):
                acc = ps2.tile([128, 512], F32, tag="s2", name="accg")
                accs[half] = acc
                nc.tensor.matmul(
                    acc,
                    wg[:, 0:128],
                    Xg[0:cg, b * N : (b + 1) * N],
                    start=True,
                    stop=False,
                )
                for m in range(1, NM):
                    for kp in range(2):
                        nc.tensor.matmul(
                            acc,
                            ag[(b, kp)][:, :, m - 1, :],
                            tm_sb["fp8"][:, m, kp],
                            start=False,
                            stop=(m == NM - 1 and kp == 1),
                            perf_mode=DR,
                        )
            # per-half acts + DVE chain: each batch's sigmoid -> r*h releases
            # that batch's cand stage-1 without waiting for the pair
            for half, b in enumerate((2 * p, 2 * p + 1)):
                bcols = slice(b * N, (b + 1) * N)
                nc.scalar.activation(
                    out=RU[:, bcols], in_=accs[half][:, :],
                    func=AF.Sigmoid, bias=bg[:, 0:1], scale=1.0 / TSCALE,
                )
                # r*h first: it gates the cand phase's stage-1 (critical path)
                nc.vector.tensor_mul(
                    out=r_dst(bcols), in0=RU[r_src, bcols], in1=h_src(bcols)
                )
            pcols = slice(2 * p * N, 2 * (p + 1) * N)
            # move u to this layer's partition rows (single-src cross-base copy)
            nc.vector.tensor_copy(out=Uu[sl, pcols], in_=RU[u_src, pcols])
            nc.vector.tensor_mul(
                out=uh[sl, pcols], in0=Uu[sl, pcols], in1=h_src(pcols)
            )
            nc.vector.tensor_scalar(
                out=Wu[sl, pcols], in0=Uu[sl, pcols],
                scalar1=-1.0, scalar2=1.0,
                op0=mybir.AluOpType.mult, op1=mybir.AluOpType.add,
            )

        def cand_s1(p):
            # stage 1: m=1..4 only; all 4 chunks of one batch share a 2-bank
            # pair tile (256 cols each) -> one wide copy per batch
            for half, b in enumerate((2 * p, 2 * p + 1)):
                pc = ps1.tile([128, 2, 512], F32, tag="s1g", name="pc")
                for k in range(KCH):
                    lhsT = Xc[0:cc, b * N + k * 128 : b * N + (k + 1) * 128]
                    nc.tensor.matmul(
                        pc[:, k // 2, (k % 2) * 256 : (k % 2) * 256 + 256],
                        lhsT,
                        wc[:, U : NM * U],
                        start=True,
                        stop=True,
                        skip_group_check=(k % 2 == 1),
                    )
                if half == 0:
                    # split A by m: m2,m4 -> bf16 (dominant), m1,m3 -> fp8 DR
                    ac[p] = (
                        ac_pool.tile([128, 2, 2, 2, 2, U], BF16, tag="a24", name="a24"),
                        ac_pool.tile([128, 2, 2, 2, 2, U], FP8, tag="a13", name="a13"),
                    )
                # pc free layout (kp, j, m, u); m = 2*m2 + mm: mm=0 -> m13,
                # mm=1 -> m24 (m index here is m-1 for m=1..4)
                srcv = pc.rearrange(
                    "p kp (j m2 mm u) -> p kp j m2 mm u", j=2, m2=2, mm=2
                )
                # balance the two casts across Scalar and Vector per half
                e24 = nc.vector if half == 0 else nc.scalar
                e13 = nc.scalar if half == 0 else nc.vector
                (e24.tensor_copy if e24 is nc.vector else e24.copy)(
                    out=ac[p][0][:, :, :, :, half, :], in_=srcv[:, :, :, :, 1, :]
                )
                (e13.tensor_copy if e13 is nc.vector else e13.copy)(
                    out=ac[p][1][:, :, :, :, half, :], in_=srcv[:, :, :, :, 0, :]
                )

        def cand_s2(p):
            # stage 2: identity fold (col-tiled pair) + diffusion, fused tanh
            # (both p's share one 2-bank pair tile, one bank each)
            acc = ps2.tile([128, 512], F32, tag="s2", name="accc")
            for half in range(2):
                b = 2 * p + half
                nc.tensor.matmul(
                    acc[half * U : (half + 1) * U, :],
                    wc[:, 0:U],
                    Xc[0:cc, b * N : (b + 1) * N],
                    start=True,  # per-partition zero region: each half starts its own rows
                    stop=False,
                    tile_position=(0, half * U),
                    # sim's group tracker isn't partition-base-aware; half 1 would
                    # falsely collide with half 0's pending group
                    skip_group_check=True,
                )
            for mi, m in enumerate((2, 4)):
                for k in range(KCH):
                    nc.tensor.matmul(
                        acc,
                        ac[p][0][:, k // 2, k % 2, mi, :, :],
                        tm_sb["bf16"][:, m, k // 2, k % 2, :],
                        start=False,
                        stop=False,
                        skip_group_check=True,
                    )
            for mi, m in enumerate((1, 3)):
                for kp in range(2):
                    nc.tensor.matmul(
                        acc,
                        ac[p][1][:, kp, :, mi, :, :],
                        tm_sb["fp8"][:, m, kp],
                        start=False,
                        stop=(mi == 1 and kp == 1),
                        perf_mode=DR,
                        skip_group_check=True,
                    )
            # per-half tanh + blend: batch b's h lands without waiting for
            # the pair, releasing the next phase's stage-1 per batch
            for half in range(2):
                b = 2 * p + half
                bcols = slice(b * N, (b + 1) * N)
                nc.scalar.activation(
                    out=Ct[sl, bcols],
                    in_=acc[half * U : (half + 1) * U, :],
                    func=AF.Tanh, bias=bc[:, 0:1], scale=1.0 / TSCALE,
                )
                # h_new = u*h + (1-u)*c
                nc.vector.tensor_mul(
                    out=wct[sl, bcols], in0=Wu[sl, bcols], in1=Ct[sl, bcols]
                )
                nc.vector.tensor_add(
                    out=h_dst(bcols), in0=uh[sl, bcols], in1=wct[sl, bcols]
                )
            post(p)

        return gate_s1, gate_s2, cand_s1, cand_s2

    def l0_h_src(cols):
        return X0[0:U, cols]

    def l0_r_dst(cols):
        return X0c[0:U, cols]

    def l0_h_dst(cols):
        # write h0' straight into X1 so g1's stage-1 doesn't wait on a copy
        return X1[0:U, cols]

    def l0_post(p):
        # fan h0' out to the other consumers, off the g1 critical path:
        # X1c feeds c1's stage-1 (medium slack), X0 feeds the NEXT step's g0
        # (GpSimd measured 5x slower than DVE for these copies)
        pcols = slice(2 * p * N, 2 * (p + 1) * N)
        nc.scalar.copy(out=X1c[0:U, pcols], in_=X1[0:U, pcols])
        nc.vector.tensor_copy(out=X0[0:U, pcols], in_=X1[0:U, pcols])

    def l1_h_src(cols):
        return X1[U:C1, cols]

    def l1_r_dst(cols):
        return X1c[U:C1, cols]

    def l1_h_dst(cols):
        return X1[U:C1, cols]

    def l1_post(p):
        pass

    def proj_phase(p, feed_cand=False):
        # projection for pair p: out = h1 . pw + pb (row 0 of pp)
        pq = {}
        for q in (2 * p, 2 * p + 1):
            pp = ps2.tile([128, 512], F32, tag="s2", name="pp")
            pq[q] = pp
            nc.tensor.matmul(
                pp[0:2, :],
                pw_sb[64:128, :],
                X1[U:C1, q * 512 : (q + 1) * 512],
                start=True,
                stop=True,
            )
        # X0 row first (gates the output DMA), X0c after (only read by the
        # later cand stage-1)
        for q in (2 * p, 2 * p + 1):
            nc.scalar.activation(
                out=X0[U:C0, q * 512 : (q + 1) * 512],
                in_=pq[q][0:1, :],
                func=AF.Identity,
                bias=pb_sb[:, 0:1],
                scale=1.0,
            )
        if feed_cand:
            # decoder feedback: write the cand-path x row directly too,
            # replacing a serial 1-partition [1, BI] copy on the DVE
            for q in (2 * p, 2 * p + 1):
                nc.scalar.activation(
                    out=X0c[U:C0, q * 512 : (q + 1) * 512],
                    in_=pq[q][0:1, :],
                    func=AF.Identity,
                    bias=pb_sb[:, 0:1],
                    scale=1.0,
                )

    def build_step(pfx, fold0=False):
        if fold0:
            # decoder t>=1: x = h1.pw (+pb==0) folded into the L0 gate
            # weights, so the gate reads [h0; h1] from X1 and never waits on
            # the projection chain
            xg0, cg0, wg0 = X1, C1, w_sb["dwg0f"]
        else:
            xg0, cg0, wg0 = X0, C0, w_sb[f"{pfx}wg0"]
        c0_cell = cell_phases(
            0, xg0, X0c, cg0, C0,
            wg0, w_sb[f"{pfx}bg0"], w_sb[f"{pfx}wc0"],
            w_sb[f"{pfx}bc0"], l0_h_src, l0_r_dst, l0_h_dst, l0_post,
        )
        c1_cell = cell_phases(
            1, X1, X1c, C1, C1,
            w_sb[f"{pfx}wg1"], w_sb[f"{pfx}bg1"], w_sb[f"{pfx}wc1"],
            w_sb[f"{pfx}bc1"], l1_h_src, l1_r_dst, l1_h_dst, l1_post,
        )
        return c0_cell, c1_cell

    def stage_x(t):
        def emit(dst):
            nc.sync.dma_start(out=dst[U:C0, :], in_=d_xenc[t : t + 1, :])
        return emit

    def zero_x(dst):
        nc.vector.memset(dst[U:C0, :], 0.0)

    # Software-pipelined emission with a 1-phase skew: the trailing cand
    # phase (and decoder projection) of step t interleaves with step t+1's
    # gate matmuls so the PE never drains at a step boundary.
    # steps: list of (pfx, x_hook or None, dec_t or None)
    steps = []
    for t in range(n_enc):
        if t == 0:
            hook = None  # x_0 staged before the loop
        else:
            hook = stage_x(t)
        steps.append(("e", hook, None))
    steps.append(("d", (lambda dst: zero_x(dst)), 0))
    for t in range(1, n_dec):
        steps.append(("d", None, t))

    stage_x(0)(X0)
    stage_x(0)(X0c)
    pending = None  # (c1, dec_t) of the previous step
    # Sub-phase schedule: s1(0), s1(1), s2(0), s2(1) per cell so pair-1's
    # stage-1 PE work always covers pair-0's PSUM->SBUF cast latency, and the
    # trailing cand-L1 stage-2 of step t runs at the head of step t+1.
    for pfx, x_hook, dec_t in steps:
        fold0 = fold and dec_t is not None and x_hook is None
        (g0s1, g0s2, c0s1, c0s2), (g1s1, g1s2, c1s1, c1s2) = build_step(
            pfx, fold0
        )
        # x for THIS step must land before this step's g0/c0 read it; the
        # hook writes row 64 only, after the previous step's readers.
        if x_hook is not None:
            x_hook(X0)
        if pending is not None:
            pc1s2, pdec = pending
            pc1s2(0); pc1s2(1)
            if pdec is not None:
                feed = dec_t is not None and x_hook is None
                proj_phase(0, feed_cand=feed)
                proj_phase(1, feed_cand=feed)
                nc.sync.dma_start(out=d_out[pdec : pdec + 1, :], in_=X0[U:C0, :])
        g0s1(0); g0s1(1)
        g0s2(0); g0s2(1)
        if x_hook is not None:
            x_hook(X0c)
        c0s1(0); c0s1(1)
        c0s2(0); c0s2(1)
        g1s1(0); g1s1(1)
        g1s2(0); g1s2(1)
        c1s1(0); c1s1(1)
        pending = (c1s2, dec_t)

    pc1s2, pdec = pending
    pc1s2(0); pc1s2(1)
    proj_phase(0); proj_phase(1)
    nc.sync.dma_start(out=d_out[pdec : pdec + 1, :], in_=X0[U:C0, :])

    for pool in (ps2, ps1, ac_pool, ag_pool, gpool, work, consts):
        pool.release()


# --------------------------------------------------------------------------
# host-side packing
# --------------------------------------------------------------------------
def _prep_shared(inputs):
    bf = mybir.dt.np(BF16)
    f8 = mybir.dt.np(FP8)
    sup = np.asarray(inputs["supports"], np.float64)
    eye = np.eye(N, dtype=np.float64)
    tms = [
        eye,
        sup[0],
        2.0 * (sup[0] @ sup[0]) - eye,
        sup[1],
        2.0 * (sup[1] @ sup[1]) - eye,
    ]
    # T (and the identity W blocks) are pre-scaled by TSCALE so fp8 entries
    # land in e4m3's normal range; the PSUM->SBUF activation undoes it via
    # scale=1/TSCALE.  |T|max ~1.05 -> 134 < 448, safe.
    tmats = np.stack([t.T * TSCALE for t in tms]).astype(np.float32)
    tmats = tmats.reshape(NM * KCH * 128, 512)

    shared = {}
    for s2 in ("fp8", "bf16"):
        dt_ = f8 if s2 == "fp8" else bf
        shared[f"tm_{s2}"] = np.ascontiguousarray(tmats.astype(dt_))
    for pfx, name in (("e", "enc"), ("d", "dec")):
        for lyr, c_in in ((0, C0), (1, C1)):
            wg = np.asarray(inputs[f"{name}{lyr}_Wg"], np.float32).reshape(
                c_in, NM * 2 * U
            )
            wc = np.asarray(inputs[f"{name}{lyr}_Wc"], np.float32).reshape(
                c_in, NM * U
            )
            bg = np.asarray(inputs[f"{name}{lyr}_bg"], np.float32)
            bc = np.asarray(inputs[f"{name}{lyr}_bc"], np.float32)
            # scale the identity (m=0) block to match the TSCALE'd T terms
            # (copy: the reshaped views alias the caller's input arrays)
            wg = wg.copy()
            wc = wc.copy()
            wg[:, 0 : 2 * U] *= TSCALE
            wc[:, 0:U] *= TSCALE
            if lyr == 0:
                perm = np.r_[1:c_in, 0]  # rows [h..., x]
                wg = wg[perm]
                wc = wc[perm]
            else:
                # layer-1 gate layout is [u; r] (see cell_phases): swap the
                # r/u column halves inside each m block, and the bias halves
                wg = np.ascontiguousarray(
                    wg.reshape(c_in, NM, 2, U)[:, :, ::-1, :].reshape(c_in, NM * 2 * U)
                )
                bg = np.concatenate([bg[U:], bg[:U]])
            shared[f"{pfx}wg{lyr}"] = np.ascontiguousarray(wg.astype(bf))
            if pfx == "d" and lyr == 0:
                pw_f = np.asarray(inputs["proj_W"], np.float64).reshape(U, 1)
                fold = np.vstack([wg[0:U], pw_f @ wg[U : U + 1]]).astype(np.float32)
                shared["dwg0f"] = np.ascontiguousarray(fold.astype(bf))
            shared[f"{pfx}wc{lyr}"] = np.ascontiguousarray(wc.astype(bf))
            shared[f"{pfx}bg{lyr}"] = np.ascontiguousarray(bg.reshape(2 * U, 1))
            shared[f"{pfx}bc{lyr}"] = np.ascontiguousarray(bc.reshape(U, 1))
    pw = np.asarray(inputs["proj_W"], np.float32).reshape(U, 1)
    shared["pw"] = np.ascontiguousarray(
        np.concatenate([pw, np.zeros((U, 1), np.float32)], axis=1).astype(bf)
    )
    shared["pb"] = np.asarray(inputs["proj_b"], np.float32).reshape(1, 1)
    return shared


def _make_in_maps(inputs, n_enc=T_ENC):
    bf = mybir.dt.np(BF16)
    shared = _prep_shared(inputs)
    x = np.asarray(inputs["inputs"], np.float32)  # (T, B, N)
    in_maps = []
    for c in range(NCORES):
        m = dict(shared)
        m["xenc"] = np.ascontiguousarray(
            x[:n_enc, c * BL : (c + 1) * BL, :].reshape(n_enc, BI).astype(bf)
        )
        in_maps.append(m)
    return in_maps


_PROG_CACHE = {}


def _get_program(n_enc=T_ENC, n_dec=HOR, fold=True):
    key = (n_enc, n_dec, fold)
    if key not in _PROG_CACHE:
        _PROG_CACHE[key] = _build_program(n_enc, n_dec, fold)
    return _PROG_CACHE[key]


def _run(inputs, n_enc=T_ENC, n_dec=HOR, **kw):
    fold = bool(np.allclose(np.asarray(inputs["proj_b"], np.float64), 0.0))
    nc = _get_program(n_enc, n_dec, fold)
    in_maps = _make_in_maps(inputs, n_enc)
    if not fold:
        for m in in_maps:
            m.pop("dwg0f", None)
    res = bass_utils.run_bass_kernel_spmd(nc, in_maps, core_ids=list(range(NCORES)), **kw)
    out = np.empty((n_dec, B, N), np.float32)
    for c in range(NCORES):
        out[:, c * BL : (c + 1) * BL, :] = (
            res.results[c]["outs"].astype(np.float32).reshape(n_dec, BL, N)
        )
    return out.reshape(n_dec, B, N), res


def kernel(**inputs) -> np.ndarray:
    out, _ = _run(inputs)
    return out.reshape(HOR, B, N)



# revision 36
# speedup vs baseline: 1.1742x; 1.0528x over previous
"""DCRNN (2-layer encoder/decoder DCGRU, N=512 nodes, B=32, U=64, K=2, 2 supports)
Trainium2 Bass/Tile kernel, data-parallel over batch across 8 NeuronCores.

Formulation: gconv(X) = sum_m T_m @ X @ W_m with T_m precomputed on host
(m=0 is the identity and is folded into stage 2 as a direct X @ W_0 matmul).
  stage 1 (dense):     A_m = X @ W_m, m=1..4  (X-chunk as lhsT -> node-major A)
  stage 2 (diffusion): out = X @ W_0 + sum_m (T_m A_m)^T, accumulated in PSUM,
                       bias+sigmoid/tanh fused into the PSUM->SBUF activation.
All matmul operands bf16 (or fp8e4m3 with DoubleRow for the diffusion stage:
two 128-row node chunks contracted per matmul at 2x rate). State tiles bf16.
Layout avoids every partition-shift copy:
  X0  [65,BI]  rows 0:64 h0,    row 64 x      (L0 gate lhsT)
  X0c [65,BI]  rows 0:64 r0*h0, row 64 x      (L0 cand lhsT)
  X1  [128,BI] rows 0:64 h0',   rows 64:128 h1        (L1 gate lhsT)
  X1c [128,BI] rows 0:64 h0',   rows 64:128 r1*h1     (L1 cand lhsT)
h1 and r1*h1 are written at partition base 64 directly by DVE (cross-base ok).
"""

import sys

sys.path.insert(0, "/opt/trn_rl_repo")

import numpy as np

import concourse.bass as bass
import concourse.mybir as mybir
import concourse.tile as tile
from concourse import bacc, bass_utils

# Model dims (fixed by the problem)
N = 512
T_ENC = 12
HOR = 12
U = 64
NM = 5  # diffusion matrices (I + 2 per support * 2 supports)
B = 32
NCORES = 8
BL = B // NCORES  # local batch = 4
BI = BL * N  # 2048: the (b, node) free dim
C0 = 1 + U  # 65 input channels, layer 0
C1 = U + U  # 128 input channels, layer 1
KCH = N // 128  # 4 node chunks

F32 = mybir.dt.float32
BF16 = mybir.dt.bfloat16
FP8 = mybir.dt.float8e4
AF = mybir.ActivationFunctionType
DR = mybir.MatmulPerfMode.DoubleRow
TSCALE = 128.0  # pre-scale on T & identity-W so fp8 T entries are normal-range

# stage-2 diffusion: gate = all-m fp8 DoubleRow; cand = m2,m4 bf16 (dominant
# terms) + m1,m3 fp8 DoubleRow (terms ~10% of output norm, fp8 error diluted)
import os as _os

LDW_OPT = _os.environ.get("LDW_OPT", "0") == "1"

_ldw_patched = False


def _patch_ldw_opt():
    global _ldw_patched
    if _ldw_patched or not LDW_OPT:
        return
    _ldw_patched = True
    orig = bass_utils.bir_verify_and_optimise

    def patched(tmpdir, inp="bir.json", outp="file.neff", arch=None, *, dve_root=None):
        import concourse.bass_utils as bu

        real_run = bu.run_command

        def run_hook(cmd, **kw):
            cmd = [
                c.replace("--enable-ldw-opt=false", "--enable-ldw-opt=true")
                for c in cmd
            ]
            return real_run(cmd, **kw)

        bu.run_command = run_hook
        try:
            return orig(tmpdir, inp, outp, arch, dve_root=dve_root)
        finally:
            bu.run_command = real_run

    bass_utils.bir_verify_and_optimise = patched


def _build_program(n_enc=T_ENC, n_dec=HOR, fold=True):
    _patch_ldw_opt()
    nc = bacc.Bacc("TRN2", target_bir_lowering=False, debug=False)

    # ---- DRAM I/O ----
    d_xenc = nc.dram_tensor("xenc", [n_enc, BI], BF16, kind="ExternalInput")
    d_tm = {}
    for s2 in ("fp8", "bf16"):
        dt_ = FP8 if s2 == "fp8" else BF16
        d_tm[s2] = nc.dram_tensor(
            f"tm_{s2}", [NM * KCH * 128, 512], dt_, kind="ExternalInput"
        )
    d_w = {}
    for pfx in ("e", "d"):
        for lyr, c_in in ((0, C0), (1, C1)):
            d_w[f"{pfx}wg{lyr}"] = nc.dram_tensor(
                f"{pfx}wg{lyr}", [c_in, NM * 2 * U], BF16, kind="ExternalInput"
            )
            d_w[f"{pfx}wc{lyr}"] = nc.dram_tensor(
                f"{pfx}wc{lyr}", [c_in, NM * U], BF16, kind="ExternalInput"
            )
            d_w[f"{pfx}bg{lyr}"] = nc.dram_tensor(
                f"{pfx}bg{lyr}", [2 * U, 1], F32, kind="ExternalInput"
            )
            d_w[f"{pfx}bc{lyr}"] = nc.dram_tensor(
                f"{pfx}bc{lyr}", [U, 1], F32, kind="ExternalInput"
            )
    if fold:
        d_w["dwg0f"] = nc.dram_tensor(
            "dwg0f", [C1, NM * 2 * U], BF16, kind="ExternalInput"
        )
    d_pw = nc.dram_tensor("pw", [U, 2], BF16, kind="ExternalInput")
    d_pb = nc.dram_tensor("pb", [1, 1], F32, kind="ExternalInput")
    d_out = nc.dram_tensor("outs", [n_dec, BI], BF16, kind="ExternalOutput")

    with tile.TileContext(nc) as tc:
        _body(tc, n_enc, n_dec, d_xenc, d_tm, d_w, d_pw, d_pb, d_out, fold)
    nc.compile()
    return nc


def _body(tc, n_enc, n_dec, d_xenc, d_tm, d_w, d_pw, d_pb, d_out, fold):
    nc = tc.nc
    consts = tc.alloc_tile_pool(name="consts", bufs=1)
    work = tc.alloc_tile_pool(name="work", bufs=1)
    gpool = tc.alloc_tile_pool(name="gpool", bufs=2)
    ag_pool = tc.alloc_tile_pool(name="agp", bufs=10)
    ac_pool = tc.alloc_tile_pool(name="acp", bufs=6)
    ps1 = tc.alloc_tile_pool(name="ps1", bufs=2, space="PSUM")
    ps2 = tc.alloc_tile_pool(name="ps2", bufs=4, space="PSUM")

    # ---- resident constants ----
    # tm layout: [128(p), m, kpair, j, 512]; node index = (kpair*2+j)*128 + p
    tm_sb = {}
    for s2 in ("fp8", "bf16"):
        dt_ = FP8 if s2 == "fp8" else BF16
        t = consts.tile([128, NM, 2, 2, 512], dt_, name=f"tm_sb_{s2}")
        tm_sb[s2] = t
        for m in range(NM):
            for k in range(KCH):
                row = (m * KCH + k) * 128
                nc.sync.dma_start(
                    out=t[:, m, k // 2, k % 2, :], in_=d_tm[s2][row : row + 128, :]
                )

    w_sb = {}
    for key, dt_ in d_w.items():
        shape = list(dt_.shape)
        sb_dt = BF16 if key[1] == "w" else F32
        w_sb[key] = consts.tile(shape, sb_dt, name=f"sb_{key}")
        nc.sync.dma_start(out=w_sb[key][:, :], in_=dt_[:, :])
    pw_sb = consts.tile([128, 2], BF16, name="pw_sb")
    nc.sync.dma_start(out=pw_sb[64:128, :], in_=d_pw[:, :])
    pb_sb = consts.tile([1, 1], F32, name="pb_sb")
    nc.sync.dma_start(out=pb_sb, in_=d_pb[:, :])

    # ---- persistent state ----
    X0 = work.tile([C0, BI], BF16, name="X0")  # [h0 ; x]
    X0c = work.tile([C0, BI], BF16, name="X0c")  # [r0*h0 ; x]
    X1 = work.tile([C1, BI], BF16, name="X1")  # [h0 ; h1]
    X1c = work.tile([C1, BI], BF16, name="X1c")  # [h0 ; r1*h1]

    nc.gpsimd.memset(X0[0:U, :], 0.0)
    nc.gpsimd.memset(X0c[0:U, :], 0.0)
    nc.gpsimd.memset(X1[:, :], 0.0)
    nc.gpsimd.memset(X1c[:, :], 0.0)

    tc.strict_bb_all_engine_barrier()

    def cell_phases(lyr, Xg, Xc, cg, cc, wg, bg, wc, bc, h_src, r_dst, h_dst, post):
        """One DCGRU cell, split into per-batch-pair phases.

        Layer-l elementwise state lives at partition base l*64 so every
        two-tensor DVE op has matching input bases (h1 sits at rows 64:128 of
        X1).  The gate output layout is [r; u] for layer 0 and [u; r] for
        layer 1 (weights pre-flipped on host), so r shares a base with h; the
        u half is moved across with one single-src copy per pair.
        """
        gdt = FP8
        sl = slice(lyr * U, (lyr + 1) * U)  # this layer's partition rows
        u_src = slice(U, 2 * U) if lyr == 0 else slice(0, U)  # u half of RU
        r_src = slice(0, U) if lyr == 0 else slice(U, 2 * U)  # r half of RU
        RU = gpool.tile([2 * U, BI], BF16, tag="RU", name="RU", bufs=2)
        Uu = gpool.tile([2 * U, BI], BF16, tag="Uu", name="Uu", bufs=2)
        Wu = gpool.tile([2 * U, BI], BF16, tag="Wu", name="Wu", bufs=2)
        uh = gpool.tile([2 * U, BI], BF16, tag="uh", name="uh", bufs=2)
        Ct = gpool.tile([2 * U, BI], BF16, tag="Ct", name="Ct", bufs=2)
        wct = gpool.tile([2 * U, BI], BF16, tag="wct", name="wct", bufs=2)
        ag = {}
        ac = {}

        def gate_s1(p):
            # stage 1: A_m = X @ Wg_m for m=1..4 (m=0 folded into stage 2).
            # Two node chunks share a 2-bank PSUM pair tile -> one wide copy.
            for b in (2 * p, 2 * p + 1):
                for kp in range(2):
                    pg = ps1.tile([128, 2, 512], F32, tag="s1g", name="pg")
                    for j in range(2):
                        k = kp * 2 + j
                        lhsT = Xg[0:cg, b * N + k * 128 : b * N + (k + 1) * 128]
                        nc.tensor.matmul(
                            pg[:, j, :], lhsT, wg[:, 128:640], start=True, stop=True
                        )
                    a = ag_pool.tile(
                        [128, 2, NM - 1, 128], gdt, tag="ag", name="ag"
                    )
                    ag[(b, kp)] = a
                    # split the PSUM->SBUF A copies across Scalar and Vector
                    # so neither queue rate-limits the gate phase
                    srcv = pg.rearrange("p j (m c) -> p j m c", m=NM - 1)
                    if (b + kp) % 2 == 0:
                        nc.scalar.copy(out=a[:, :, :, :], in_=srcv)
                    else:
                        nc.vector.tensor_copy(out=a[:, :, :, :], in_=srcv)

        def gate_s2(p):
            # stage 2: acc = X @ Wg_0 + sum_{m>0} (T_m A_m)^T, fused sigmoid.
            # Per-half 1-bank acc tiles keep the ps2 ring loosely coupled.
            accs = {}
            for half, b in enumerate((2 * p, 2 * p + 1)):
                acc = ps2.tile([128, 512], F32, tag="s2", name="accg")
                accs[half] = acc
                nc.tensor.matmul(
                    acc,
                    wg[:, 0:128],
                    Xg[0:cg, b * N : (b + 1) * N],
                    start=True,
                    stop=False,
                )
                for m in range(1, NM):
                    for kp in range(2):
                        nc.tensor.matmul(
                            acc,
                            ag[(b, kp)][:, :, m - 1, :],
                            tm_sb["fp8"][:, m, kp],
                            start=False,
                            stop=(m == NM - 1 and kp == 1),
                            perf_mode=DR,
                        )
            # per-half acts + DVE chain: each batch's sigmoid -> r*h releases
            # that batch's cand stage-1 without waiting for the pair
            for half, b in enumerate((2 * p, 2 * p + 1)):
                bcols = slice(b * N, (b + 1) * N)
                nc.scalar.activation(
                    out=RU[:, bcols], in_=accs[half][:, :],
                    func=AF.Sigmoid, bias=bg[:, 0:1], scale=1.0 / TSCALE,
                )
                # r*h first: it gates the cand phase's stage-1 (critical path)
                nc.vector.tensor_mul(
                    out=r_dst(bcols), in0=RU[r_src, bcols], in1=h_src(bcols)
                )
            pcols = slice(2 * p * N, 2 * (p + 1) * N)
            # move u to this layer's partition rows (single-src cross-base copy)
            nc.vector.tensor_copy(out=Uu[sl, pcols], in_=RU[u_src, pcols])
            nc.vector.tensor_mul(
                out=uh[sl, pcols], in0=Uu[sl, pcols], in1=h_src(pcols)
            )
            nc.vector.tensor_scalar(
                out=Wu[sl, pcols], in0=Uu[sl, pcols],
                scalar1=-1.0, scalar2=1.0,
                op0=mybir.AluOpType.mult, op1=mybir.AluOpType.add,
            )

        def cand_s1(p):
            # stage 1: m=1..4 only; all 4 chunks of one batch share a 2-bank
            # pair tile (256 cols each) -> one wide copy per batch
            for half, b in enumerate((2 * p, 2 * p + 1)):
                pc = ps1.tile([128, 2, 512], F32, tag="s1g", name="pc")
                for k in range(KCH):
                    lhsT = Xc[0:cc, b * N + k * 128 : b * N + (k + 1) * 128]
                    nc.tensor.matmul(
                        pc[:, k // 2, (k % 2) * 256 : (k % 2) * 256 + 256],
                        lhsT,
                        wc[:, U : NM * U],
                        start=True,
                        stop=True,
                        skip_group_check=(k % 2 == 1),
                    )
                if half == 0:
                    # split A by m: m2,m4 -> bf16 (dominant), m1,m3 -> fp8 DR
                    ac[p] = (
                        ac_pool.tile([128, 2, 2, 2, 2, U], BF16, tag="a24", name="a24"),
                        ac_pool.tile([128, 2, 2, 2, 2, U], FP8, tag="a13", name="a13"),
                    )
                # pc free layout (kp, j, m, u); m = 2*m2 + mm: mm=0 -> m13,
                # mm=1 -> m24 (m index here is m-1 for m=1..4)
                srcv = pc.rearrange(
                    "p kp (j m2 mm u) -> p kp j m2 mm u", j=2, m2=2, mm=2
                )
                # balance the two casts across Scalar and Vector per half
                e24 = nc.vector if half == 0 else nc.scalar
                e13 = nc.scalar if half == 0 else nc.vector
                (e24.tensor_copy if e24 is nc.vector else e24.copy)(
                    out=ac[p][0][:, :, :, :, half, :], in_=srcv[:, :, :, :, 1, :]
                )
                (e13.tensor_copy if e13 is nc.vector else e13.copy)(
                    out=ac[p][1][:, :, :, :, half, :], in_=srcv[:, :, :, :, 0, :]
                )

        def cand_s2(p):
            # stage 2: identity fold (col-tiled pair) + diffusion, fused tanh
            # (both p's share one 2-bank pair tile, one bank each)
            acc = ps2.tile([128, 512], F32, tag="s2", name="accc")
            for half in range(2):
                b = 2 * p + half
                nc.tensor.matmul(
                    acc[half * U : (half + 1) * U, :],
                    wc[:, 0:U],
                    Xc[0:cc, b * N : (b + 1) * N],
                    start=True,  # per-partition zero region: each half starts its own rows
                    stop=False,
                    tile_position=(0, half * U),
                    # sim's group tracker isn't partition-base-aware; half 1 would
                    # falsely collide with half 0's pending group
                    skip_group_check=True,
                )
            for mi, m in enumerate((2, 4)):
                for k in range(KCH):
                    nc.tensor.matmul(
                        acc,
                        ac[p][0][:, k // 2, k % 2, mi, :, :],
                        tm_sb["bf16"][:, m, k // 2, k % 2, :],
                        start=False,
                        stop=False,
                        skip_group_check=True,
                    )
            for mi, m in enumerate((1, 3)):
                for kp in range(2):
                    nc.tensor.matmul(
                        acc,
                        ac[p][1][:, kp, :, mi, :, :],
                        tm_sb["fp8"][:, m, kp],
                        start=False,
                        stop=(mi == 1 and kp == 1),
                        perf_mode=DR,
                        skip_group_check=True,
                    )
            # per-half tanh + blend: batch b's h lands without waiting for
            # the pair, releasing the next phase's stage-1 per batch
            for half in range(2):
                b = 2 * p + half
                bcols = slice(b * N, (b + 1) * N)
                nc.scalar.activation(
                    out=Ct[sl, bcols],
                    in_=acc[half * U : (half + 1) * U, :],
                    func=AF.Tanh, bias=bc[:, 0:1], scale=1.0 / TSCALE,
                )
                # h_new = u*h + (1-u)*c
                nc.vector.tensor_mul(
                    out=wct[sl, bcols], in0=Wu[sl, bcols], in1=Ct[sl, bcols]
                )
                nc.vector.tensor_add(
                    out=h_dst(bcols), in0=uh[sl, bcols], in1=wct[sl, bcols]
                )
                post(bcols)

        return gate_s1, gate_s2, cand_s1, cand_s2

    def l0_h_src(cols):
        return X0[0:U, cols]

    def l0_r_dst(cols):
        return X0c[0:U, cols]

    def l0_h_dst(cols):
        # write h0' straight into X1 so g1's stage-1 doesn't wait on a copy
        return X1[0:U, cols]

    def l0_post(bcols):
        # fan h0' out per half right after its blend: DVE COPY runs in 2x
        # mode (~0.3us), so X1c (read by c1's stage-1) is ready fast; X0
        # only feeds the NEXT step's g0
        nc.vector.tensor_copy(out=X1c[0:U, bcols], in_=X1[0:U, bcols])
        nc.vector.tensor_copy(out=X0[0:U, bcols], in_=X1[0:U, bcols])

    def l1_h_src(cols):
        return X1[U:C1, cols]

    def l1_r_dst(cols):
        return X1c[U:C1, cols]

    def l1_h_dst(cols):
        return X1[U:C1, cols]

    def l1_post(bcols):
        pass

    def proj_phase(p, feed_cand=False):
        # projection for pair p: out = h1 . pw + pb (row 0 of pp)
        pq = {}
        for q in (2 * p, 2 * p + 1):
            pp = ps2.tile([128, 512], F32, tag="s2", name="pp")
            pq[q] = pp
            nc.tensor.matmul(
                pp[0:2, :],
                pw_sb[64:128, :],
                X1[U:C1, q * 512 : (q + 1) * 512],
                start=True,
                stop=True,
            )
        # X0 row first (gates the output DMA), X0c after (only read by the
        # later cand stage-1)
        for q in (2 * p, 2 * p + 1):
            nc.scalar.activation(
                out=X0[U:C0, q * 512 : (q + 1) * 512],
                in_=pq[q][0:1, :],
                func=AF.Identity,
                bias=pb_sb[:, 0:1],
                scale=1.0,
            )
        if feed_cand:
            # decoder feedback: write the cand-path x row directly too,
            # replacing a serial 1-partition [1, BI] copy on the DVE
            for q in (2 * p, 2 * p + 1):
                nc.scalar.activation(
                    out=X0c[U:C0, q * 512 : (q + 1) * 512],
                    in_=pq[q][0:1, :],
                    func=AF.Identity,
                    bias=pb_sb[:, 0:1],
                    scale=1.0,
                )

    def build_step(pfx, fold0=False):
        if fold0:
            # decoder t>=1: x = h1.pw (+pb==0) folded into the L0 gate
            # weights, so the gate reads [h0; h1] from X1 and never waits on
            # the projection chain
            xg0, cg0, wg0 = X1, C1, w_sb["dwg0f"]
        else:
            xg0, cg0, wg0 = X0, C0, w_sb[f"{pfx}wg0"]
        c0_cell = cell_phases(
            0, xg0, X0c, cg0, C0,
            wg0, w_sb[f"{pfx}bg0"], w_sb[f"{pfx}wc0"],
            w_sb[f"{pfx}bc0"], l0_h_src, l0_r_dst, l0_h_dst, l0_post,
        )
        c1_cell = cell_phases(
            1, X1, X1c, C1, C1,
            w_sb[f"{pfx}wg1"], w_sb[f"{pfx}bg1"], w_sb[f"{pfx}wc1"],
            w_sb[f"{pfx}bc1"], l1_h_src, l1_r_dst, l1_h_dst, l1_post,
        )
        return c0_cell, c1_cell

    def stage_x(t):
        def emit(dst):
            nc.sync.dma_start(out=dst[U:C0, :], in_=d_xenc[t : t + 1, :])
        return emit

    def zero_x(dst):
        nc.vector.memset(dst[U:C0, :], 0.0)

    # Software-pipelined emission with a 1-phase skew: the trailing cand
    # phase (and decoder projection) of step t interleaves with step t+1's
    # gate matmuls so the PE never drains at a step boundary.
    # steps: list of (pfx, x_hook or None, dec_t or None)
    steps = []
    for t in range(n_enc):
        if t == 0:
            hook = None  # x_0 staged before the loop
        else:
            hook = stage_x(t)
        steps.append(("e", hook, None))
    steps.append(("d", (lambda dst: zero_x(dst)), 0))
    for t in range(1, n_dec):
        steps.append(("d", None, t))

    stage_x(0)(X0)
    stage_x(0)(X0c)
    pending = None  # (c1, dec_t) of the previous step
    # Sub-phase schedule: s1(0), s1(1), s2(0), s2(1) per cell so pair-1's
    # stage-1 PE work always covers pair-0's PSUM->SBUF cast latency, and the
    # trailing cand-L1 stage-2 of step t runs at the head of step t+1.
    for pfx, x_hook, dec_t in steps:
        fold0 = fold and dec_t is not None and x_hook is None
        (g0s1, g0s2, c0s1, c0s2), (g1s1, g1s2, c1s1, c1s2) = build_step(
            pfx, fold0
        )
        # x for THIS step must land before this step's g0/c0 read it; the
        # hook writes row 64 only, after the previous step's readers.
        if x_hook is not None:
            x_hook(X0)
        if pending is not None:
            pc1s2, pdec = pending
            pc1s2(0); pc1s2(1)
            if pdec is not None:
                feed = dec_t is not None and x_hook is None
                proj_phase(0, feed_cand=feed)
                proj_phase(1, feed_cand=feed)
                nc.sync.dma_start(out=d_out[pdec : pdec + 1, :], in_=X0[U:C0, :])
        g0s1(0); g0s1(1)
        g0s2(0); g0s2(1)
        if x_hook is not None:
            x_hook(X0c)
        c0s1(0); c0s1(1)
        c0s2(0); c0s2(1)
        g1s1(0); g1s1(1)
        g1s2(0); g1s2(1)
        c1s1(0); c1s1(1)
        pending = (c1s2, dec_t)

    pc1s2, pdec = pending
    pc1s2(0); pc1s2(1)
    proj_phase(0); proj_phase(1)
    nc.sync.dma_start(out=d_out[pdec : pdec + 1, :], in_=X0[U:C0, :])

    for pool in (ps2, ps1, ac_pool, ag_pool, gpool, work, consts):
        pool.release()


# --------------------------------------------------------------------------
# host-side packing
# --------------------------------------------------------------------------
def _prep_shared(inputs):
    bf = mybir.dt.np(BF16)
    f8 = mybir.dt.np(FP8)
    sup = np.asarray(inputs["supports"], np.float64)
    eye = np.eye(N, dtype=np.float64)
    tms = [
        eye,
        sup[0],
        2.0 * (sup[0] @ sup[0]) - eye,
        sup[1],
        2.0 * (sup[1] @ sup[1]) - eye,
    ]
    # T (and the identity W blocks) are pre-scaled by TSCALE so fp8 entries
    # land in e4m3's normal range; the PSUM->SBUF activation undoes it via
    # scale=1/TSCALE.  |T|max ~1.05 -> 134 < 448, safe.
    tmats = np.stack([t.T * TSCALE for t in tms]).astype(np.float32)
    tmats = tmats.reshape(NM * KCH * 128, 512)

    shared = {}
    for s2 in ("fp8", "bf16"):
        dt_ = f8 if s2 == "fp8" else bf
        shared[f"tm_{s2}"] = np.ascontiguousarray(tmats.astype(dt_))
    for pfx, name in (("e", "enc"), ("d", "dec")):
        for lyr, c_in in ((0, C0), (1, C1)):
            wg = np.asarray(inputs[f"{name}{lyr}_Wg"], np.float32).reshape(
                c_in, NM * 2 * U
            )
            wc = np.asarray(inputs[f"{name}{lyr}_Wc"], np.float32).reshape(
                c_in, NM * U
            )
            bg = np.asarray(inputs[f"{name}{lyr}_bg"], np.float32)
            bc = np.asarray(inputs[f"{name}{lyr}_bc"], np.float32)
            # scale the identity (m=0) block to match the TSCALE'd T terms
            # (copy: the reshaped views alias the caller's input arrays)
            wg = wg.copy()
            wc = wc.copy()
            wg[:, 0 : 2 * U] *= TSCALE
            wc[:, 0:U] *= TSCALE
            if lyr == 0:
                perm = np.r_[1:c_in, 0]  # rows [h..., x]
                wg = wg[perm]
                wc = wc[perm]
            else:
                # layer-1 gate layout is [u; r] (see cell_phases): swap the
                # r/u column halves inside each m block, and the bias halves
                wg = np.ascontiguousarray(
                    wg.reshape(c_in, NM, 2, U)[:, :, ::-1, :].reshape(c_in, NM * 2 * U)
                )
                bg = np.concatenate([bg[U:], bg[:U]])
            shared[f"{pfx}wg{lyr}"] = np.ascontiguousarray(wg.astype(bf))
            if pfx == "d" and lyr == 0:
                pw_f = np.asarray(inputs["proj_W"], np.float64).reshape(U, 1)
                fold = np.vstack([wg[0:U], pw_f @ wg[U : U + 1]]).astype(np.float32)
                shared["dwg0f"] = np.ascontiguousarray(fold.astype(bf))
            shared[f"{pfx}wc{lyr}"] = np.ascontiguousarray(wc.astype(bf))
            shared[f"{pfx}bg{lyr}"] = np.ascontiguousarray(bg.reshape(2 * U, 1))
            shared[f"{pfx}bc{lyr}"] = np.ascontiguousarray(bc.reshape(U, 1))
    pw = np.asarray(inputs["proj_W"], np.float32).reshape(U, 1)
    shared["pw"] = np.ascontiguousarray(
        np.concatenate([pw, np.zeros((U, 1), np.float32)], axis=1).astype(bf)
    )
    shared["pb"] = np.asarray(inputs["proj_b"], np.float32).reshape(1, 1)
    return shared


def _make_in_maps(inputs, n_enc=T_ENC):
    bf = mybir.dt.np(BF16)
    shared = _prep_shared(inputs)
    x = np.asarray(inputs["inputs"], np.float32)  # (T, B, N)
    in_maps = []
    for c in range(NCORES):
        m = dict(shared)
        m["xenc"] = np.ascontiguousarray(
            x[:n_enc, c * BL : (c + 1) * BL, :].reshape(n_enc, BI).astype(bf)
        )
        in_maps.append(m)
    return in_maps


_PROG_CACHE = {}


def _get_program(n_enc=T_ENC, n_dec=HOR, fold=True):
    key = (n_enc, n_dec, fold)
    if key not in _PROG_CACHE:
        _PROG_CACHE[key] = _build_program(n_enc, n_dec, fold)
    return _PROG_CACHE[key]


def _run(inputs, n_enc=T_ENC, n_dec=HOR, **kw):
    fold = bool(np.allclose(np.asarray(inputs["proj_b"], np.float64), 0.0))
    nc = _get_program(n_enc, n_dec, fold)
    in_maps = _make_in_maps(inputs, n_enc)
    if not fold:
        for m in in_maps:
            m.pop("dwg0f", None)
    res = bass_utils.run_bass_kernel_spmd(nc, in_maps, core_ids=list(range(NCORES)), **kw)
    out = np.empty((n_dec, B, N), np.float32)
    for c in range(NCORES):
        out[:, c * BL : (c + 1) * BL, :] = (
            res.results[c]["outs"].astype(np.float32).reshape(n_dec, BL, N)
        )
    return out.reshape(n_dec, B, N), res


def kernel(**inputs) -> np.ndarray:
    out, _ = _run(inputs)
    return out.reshape(HOR, B, N)

